# revision 29
# baseline (speedup 1.0000x reference)
"""GCN 2-layer (PyG GCNConv x2 + ReLU) Bass kernel for Trainium2, 8-core SPMD.

Strategy:
  - Host: compute symmetric normalization dinv = deg^-1/2 (self-loop incl.),
    fold dinv[src] into a prescaled gather table (x * dinv).  Dst nodes are
    RELABELED via degree-balanced LPT binning into 8 cores x 49 windows of
    128 nodes, so every (core, window) has a near-equal edge count and chunk
    padding is <1%.  Per core, edges sorted by dst window and packed into
    128-edge chunks (one matmul each).  dma_gather uses int16 indices, so
    the node table is addressed via two 32768-row views (LOW/HIGH); srcs in
    the overlap may go to either section and are assigned to fill chunk
    boundaries exactly.  Each window's LOW and HIGH chunks run back to
    back so its PSUM accumulation closes in one pass (single eviction,
    epilogues spread across the phase).  Biases are folded away when zero
    (relu(dinv*x) = dinv*relu(x); dinv^2 in the final replicate-scale).
    Self-loop edges are NOT gathered: the permuted
    table makes each window's own rows contiguous, so they stream in with a
    plain DMA and join the accumulation as one identity-operand matmul.
  - Device per core:
      Gathers are issued round-robin on the 4 SWDGE queues (num_swdge_
      queues=4).  Descriptor generation for queues 1-3 runs on their own Q7
      core pairs concurrently under queue 0's engine-blocking gather,
      cutting effective desc-gen from 8.4 ns/idx to ~2.6 ns/idx.
      Phase A (layer 1): dma_gather source rows of the prescaled x-table ->
        G [128e, d_in]; build one-hot S [128e, 128dst] on DVE (bf16 iota ==
        dst_rel); PE matmul accumulates G.T @ S into PSUM [d_in, 128dst]
        per window.  Per window: x W1 (PE), scale by dinv[dst], +b1, ReLU;
        transpose (PE); x W2; scale by dinv[dst]; replicate 64x -> 256B
        rows of the h2 table, DMA out.
      AllGather h2 shards -> full replicated table.
      Phase B (layer 2): same chunk structure; gather h2 rows, matmul
        S.T @ G2[:, :2] accumulated per window; scale by dinv[dst], +b2.
    Output rows are in permuted order; the host scatters them back.
"""

import numpy as np

import concourse.bass as bass
import concourse.mybir as mybir
import concourse.tile as tile
from concourse import bacc
from concourse.bass_utils import run_bass_kernel_spmd

F32 = mybir.dt.float32
BF16 = mybir.dt.bfloat16
I16 = mybir.dt.int16

N_CORES = 8
WINDOW = 128  # dst nodes per PSUM accumulation window
CHUNK = 128  # edges per matmul chunk
GSZ = 8  # max chunks per dma_gather instruction (1024 idxs, single-packet)
SBATCH = 8  # chunks per S-build DVE op
HALF = 32768  # int16 index range
REP = 64  # h2 replication (64x2 bf16 cols -> 256B rows)
W_SPLITS = (0, 25, 49)  # h2 AllGather pieces: window ranges, exchanged as each closes
GATHER_BF16 = True  # layer-1 gather table + chunk matmuls in bf16


# --------------------------------------------------------------------------
# Host preprocessing
# --------------------------------------------------------------------------
def _preprocess(x, edge_index, n_cores):
    N = x.shape[0]
    src = np.asarray(edge_index[0], dtype=np.int64)
    dst = np.asarray(edge_index[1], dtype=np.int64)
    # degree includes the self-loop (handled separately on-device)
    deg = (np.bincount(dst, minlength=N) + 1).astype(np.float64)
    dinv = np.where(deg > 0, 1.0 / np.sqrt(deg), 0.0).astype(np.float32)

    # ---- degree-balanced dst binning: 392 bins (8 cores x 49 windows), ----
    # ---- capacity 128 nodes each; LPT greedy on in-degree ----
    import heapq

    w_cnt = 49
    n_bins = n_cores * w_cnt
    n_local = w_cnt * WINDOW  # 6272 (padded shard; empty slots get dinv 0)
    deg_ns = np.bincount(dst, minlength=N).astype(np.float64)
    order_nodes = np.argsort(-deg_ns, kind="stable")
    heap = [(0.0, b, 0) for b in range(n_bins)]  # (edge_sum, bin, n_nodes)
    heapq.heapify(heap)
    bin_of = np.empty(N, dtype=np.int64)
    slot_of = np.empty(N, dtype=np.int64)
    stash = []
    for n in order_nodes:
        while True:
            s_, b, cnt = heapq.heappop(heap)
            if cnt < WINDOW:
                break
            stash.append((s_, b, cnt))  # full bin, drop
        bin_of[n] = b
        slot_of[n] = cnt
        heapq.heappush(heap, (s_ + deg_ns[n], b, cnt + 1))
    # permuted position of node n
    permpos = bin_of * WINDOW + slot_of  # in [0, 50176)
    # inverse: node at permuted position p (or -1)
    inv = np.full(n_cores * n_local, -1, dtype=np.int64)
    inv[permpos] = np.arange(N)

    # table rows: 0 = zero, 1..NP = permuted slots, NP+1 = zero.
    # row(node n) = permpos[n] + 1
    # LOW view = rows [0, min(HALF, NP+2)); HIGH view = rows [HB, HB+HALF)
    NP = n_cores * n_local
    HB = max(0, NP + 2 - HALF)
    lowmax_row = min(HALF, NP + 2)
    pad_low = 0
    pad_high = NP + 1 - HB

    # table layout: row 0 = zero; A-region rows [1, 1+nc*ASZ) hold the
    # first A_WIN windows of every shard (early AllGather); B-region rows
    # hold the rest; last row = zero.  rowof(p) maps permuted position.
    p_c = permpos // n_local
    p_off = permpos % n_local
    tabrow_of_pos = np.zeros(N, dtype=np.int64)
    roff = 1
    for r in range(len(W_SPLITS) - 1):
        lo, hi = W_SPLITS[r] * WINDOW, W_SPLITS[r + 1] * WINDOW
        rsz = hi - lo
        m = (p_off >= lo) & (p_off < hi)
        tabrow_of_pos[m] = roff + p_c[m] * rsz + (p_off[m] - lo)
        roff += n_cores * rsz
    rows_all = tabrow_of_pos[src]  # gather row per edge
    dstbin = bin_of[dst]
    dstslot = slot_of[dst]

    # group edges by dst bin
    eorder = np.argsort(dstbin, kind="stable")
    g_rows = rows_all[eorder]
    g_slot = dstslot[eorder]
    g_bin = dstbin[eorder]
    starts = np.searchsorted(g_bin, np.arange(n_bins + 1))

    # per (core, window): split edges into LOW/HIGH with flex edges
    # (rows in [HB, lowmax) fit either view). kw must be uniform across
    # cores, so pick per-window section targets jointly: kw_lo = max_c
    # ceil(lo_only/128), all cores fill LOW to that boundary with flex.
    split = {}
    for b in range(n_bins):
        c, w = divmod(b, w_cnt)
        rows = g_rows[starts[b] : starts[b + 1]]
        rel = g_slot[starts[b] : starts[b + 1]].astype(np.float32)
        is_lo_only = rows < HB
        is_hi_only = rows >= lowmax_row
        is_flex = ~is_lo_only & ~is_hi_only
        split[(c, w)] = (
            (rows[is_lo_only], rel[is_lo_only]),
            (rows[is_hi_only], rel[is_hi_only]),
            (rows[is_flex], rel[is_flex]),
        )
    parts = {}
    kw_lo = np.zeros(w_cnt, dtype=np.int64)
    kw_hi = np.zeros(w_cnt, dtype=np.int64)
    for w in range(w_cnt):
        klo_a = max(
            1, max(-(-len(split[(c, w)][0][0]) // CHUNK) for c in range(n_cores))
        )
        khi_a = max(
            -(
                -(len(split[(c, w)][1][0]) + max(
                    0,
                    len(split[(c, w)][2][0])
                    - (klo_a * CHUNK - len(split[(c, w)][0][0])),
                ))
                // CHUNK
            )
            for c in range(n_cores)
        )
        khi_a = max(1, khi_a)
        # alternative: minimize HIGH first
        khi_b = max(
            1, max(-(-len(split[(c, w)][1][0]) // CHUNK) for c in range(n_cores))
        )
        klo_b = max(
            -(
                -(len(split[(c, w)][0][0]) + max(
                    0,
                    len(split[(c, w)][2][0])
                    - (khi_b * CHUNK - len(split[(c, w)][1][0])),
                ))
                // CHUNK
            )
            for c in range(n_cores)
        )
        klo_b = max(1, klo_b)
        if klo_a + khi_a <= klo_b + khi_b:
            kw_lo[w], kw_hi[w], fill_low = klo_a, khi_a, True
        else:
            kw_lo[w], kw_hi[w], fill_low = klo_b, khi_b, False
        for c in range(n_cores):
            (lo_r, lo_s), (hi_r, hi_s), (fx_r, fx_s) = split[(c, w)]
            if fill_low:
                take = min(len(fx_r), kw_lo[w] * CHUNK - len(lo_r))
            else:
                take = len(fx_r) - min(len(fx_r), kw_hi[w] * CHUNK - len(hi_r))
            parts[(c, w, 0)] = (
                np.concatenate([lo_r, fx_r[:take]]),
                np.concatenate([lo_s, fx_s[:take]]),
            )
            parts[(c, w, 1)] = (
                np.concatenate([hi_r, fx_r[take:]]) - HB,
                np.concatenate([hi_s, fx_s[take:]]),
            )
    T_lo, T_hi = int(kw_lo.sum()), int(kw_hi.sum())
    T = T_lo + T_hi

    # chunk order: per window, LOW chunks then HIGH chunks (PSUM stays
    # open across both sections; one eviction per window).
    chunk_win = []  # (window, first_overall, last_overall, section)
    for w in range(w_cnt):
        kt = int(kw_lo[w] + kw_hi[w])
        k = 0
        for sec, kws in ((0, kw_lo), (1, kw_hi)):
            for _ in range(int(kws[w])):
                chunk_win.append((w, k == 0, k == kt - 1, sec))
                k += 1

    per_core = []
    for c in range(n_cores):
        idx_lin = np.zeros(T * CHUNK, dtype=np.int32)
        dstrel = np.zeros((CHUNK, T), dtype=np.float32)
        t = 0
        for w in range(w_cnt):
            for sec, kws, padrow in ((0, kw_lo, pad_low), (1, kw_hi, pad_high)):
                rows, rel = parts[(c, w, sec)]
                n_e = len(rows)
                n_slots = int(kws[w]) * CHUNK
                buf = np.full(n_slots, padrow, dtype=np.int32)
                buf[:n_e] = rows
                idx_lin[t * CHUNK : t * CHUNK + n_slots] = buf
                rbuf = np.zeros(n_slots, dtype=np.float32)
                rbuf[:n_e] = rel
                dstrel[:, t : t + int(kws[w])] = rbuf.reshape(int(kws[w]), CHUNK).T
                t += int(kws[w])
        assert t == T
        idx16 = idx_lin.astype(np.int16).reshape(T * CHUNK // 16, 16).T  # [16, S]
        idx16 = np.tile(idx16, (8, 1))  # [128, S]

        dinvw = np.zeros((WINDOW, w_cnt), dtype=np.float32)
        for w in range(w_cnt):
            nodes = inv[(c * w_cnt + w) * WINDOW : (c * w_cnt + w + 1) * WINDOW]
            valid = nodes >= 0
            dinvw[valid, w] = dinv[nodes[valid]]
        per_core.append({"idx16": idx16, "dstrel": dstrel, "dinvw": dinvw})

    kw_tot = kw_lo + kw_hi
    t_agsplit = [int(kw_tot[: W_SPLITS[r + 1]].sum()) for r in range(len(W_SPLITS) - 2)]

    return {
        "t_agsplit": t_agsplit,
        "tabrow_of_pos": tabrow_of_pos,
        "n_local": n_local,
        "w_cnt": w_cnt,
        "kw_lo": kw_lo,
        "kw_hi": kw_hi,
        "T_lo": T_lo,
        "T_hi": T_hi,
        "T": T,
        "HB": HB,
        "chunk_win": chunk_win,
        "dinv": dinv,
        "permpos": permpos,
        "inv": inv,
        "per_core": per_core,
    }


# --------------------------------------------------------------------------
# Device kernel builder (one program, SPMD across cores)
# --------------------------------------------------------------------------
def _build(nc, *, N, n_local, d_in, d_hid, n_cls, pp, n_cores, dt_gat, bias_zero, bias2_zero):
    Relu = mybir.ActivationFunctionType.Relu
    Copy = mybir.ActivationFunctionType.Copy
    T, T_lo = pp["T"], pp["T_lo"]
    w_cnt, HB = pp["w_cnt"], pp["HB"]
    chunk_win = pp["chunk_win"]
    d_rep = REP * n_cls  # 64 cols of f32 -> 256B rows

    xtab = nc.dram_tensor("xtab", [N + 2, d_in], dt_gat, kind="ExternalInput")
    w1 = nc.dram_tensor("w1", [d_in, d_hid], F32, kind="ExternalInput")
    w2 = nc.dram_tensor("w2", [d_hid, n_cls], F32, kind="ExternalInput")
    b1bc = nc.dram_tensor("b1bc", [WINDOW, d_hid], F32, kind="ExternalInput")
    b2bc = nc.dram_tensor("b2bc", [WINDOW, n_cls], F32, kind="ExternalInput")
    iota = nc.dram_tensor("iota", [CHUNK, SBATCH * WINDOW], BF16, kind="ExternalInput")
    ident = nc.dram_tensor("ident", [WINDOW, WINDOW], F32, kind="ExternalInput")
    identb = nc.dram_tensor("identb", [WINDOW, WINDOW], dt_gat, kind="ExternalInput")
    xself = nc.dram_tensor("xself", [n_local, d_in], dt_gat, kind="ExternalInput")
    idx_t = nc.dram_tensor("idx16", [CHUNK, T * 8], I16, kind="ExternalInput")
    dstrel_t = nc.dram_tensor("dstrel", [CHUNK, T], BF16, kind="ExternalInput")
    dinvw_t = nc.dram_tensor("dinvw", [WINDOW, w_cnt], F32, kind="ExternalInput")
    out_t = nc.dram_tensor("out", [n_local, n_cls], F32, kind="ExternalOutput")

    h2loc = nc.dram_tensor("h2loc", [n_local, d_rep], BF16)
    h2tab = nc.dram_tensor("h2tab", [N + 2, d_rep], BF16, addr_space="Shared")

    # gather groups: runs of chunks sharing (window, section), split into
    # near-equal pieces of <= GSZ; round-robin across the 4 SWDGE queues.
    groups = []
    qn = 0
    t0 = 0
    while t0 < T:
        w0, _, _, sec0 = chunk_win[t0]
        t1 = t0
        while t1 < T and chunk_win[t1][0] == w0 and chunk_win[t1][3] == sec0:
            t1 += 1
        run = t1 - t0
        n_pieces = -(-run // GSZ)
        base = run // n_pieces
        rem = run % n_pieces
        tt0 = t0
        for i in range(n_pieces):
            n = base + (1 if i < rem else 0)
            groups.append((sec0, tt0, n, qn % 4))
            qn += 1
            tt0 += n
        t0 = t1

    # (row offset into h2tab/xtab, local row range) per AllGather piece
    regions = []
    roff = 1
    for r in range(len(W_SPLITS) - 1):
        lo, hi = W_SPLITS[r] * WINDOW, W_SPLITS[r + 1] * WINDOW
        regions.append((roff, lo, hi))
        roff += n_cores * (hi - lo)
    t_agsplit = pp["t_agsplit"]

    def emit_exchange(r):
        roff_r, lo, hi = regions[r]
        if n_cores > 1:
            nc.gpsimd.collective_compute(
                "AllGather",
                mybir.AluOpType.bypass,
                replica_groups=[list(range(n_cores))],
                ins=[h2loc[lo:hi, :]],
                outs=[h2tab[roff_r : roff_r + n_cores * (hi - lo), :]],
            )
        else:
            nc.sync.dma_start(
                out=h2tab[roff_r : roff_r + (hi - lo), :], in_=h2loc[lo:hi, :]
            )

    def tab_view(tab):
        return [
            tab[0 : min(HALF, N + 2), :],
            tab[HB : min(HB + HALF, N + 2), :],
        ]

    with tile.TileContext(nc) as tc:
        with (
            tc.tile_pool(name="const", bufs=1) as cpool,
            tc.tile_pool(name="gbuf", bufs=12) as gpool,
            tc.tile_pool(name="g2buf", bufs=12) as g2pool,
            tc.tile_pool(name="sbat", bufs=8) as spool,
            tc.tile_pool(name="sbat2", bufs=8) as s2pool,
            tc.tile_pool(name="wtmp", bufs=3) as wpool,
            tc.tile_pool(name="selfp", bufs=3) as selfpool,
            tc.tile_pool(name="psA", bufs=4, space="PSUM") as psA,
            tc.tile_pool(name="psW", bufs=4, space="PSUM") as psW,
        ):
            # ---- constants into SBUF ----
            w1_sb = cpool.tile([d_in, d_hid], F32, tag="w1")
            nc.sync.dma_start(out=w1_sb[:], in_=w1[:])
            w2_sb = cpool.tile([d_hid, n_cls], F32, tag="w2")
            nc.sync.dma_start(out=w2_sb[:], in_=w2[:])
            b1_sb = cpool.tile([WINDOW, d_hid], F32, tag="b1")
            nc.sync.dma_start(out=b1_sb[:], in_=b1bc[:])
            b2_sb = cpool.tile([WINDOW, n_cls], F32, tag="b2")
            nc.sync.dma_start(out=b2_sb[:], in_=b2bc[:])
            iota_sb = cpool.tile([CHUNK, SBATCH * WINDOW], BF16, tag="iota")
            nc.sync.dma_start(out=iota_sb[:], in_=iota[:])
            id_sb = cpool.tile([WINDOW, WINDOW], F32, tag="ident")
            nc.sync.dma_start(out=id_sb[:], in_=ident[:])
            idb_sb = cpool.tile([WINDOW, WINDOW], dt_gat, tag="identb")
            nc.sync.dma_start(out=idb_sb[:], in_=identb[:])
            idx_sb = cpool.tile([CHUNK, T * 8], I16, tag="idx")
            nc.sync.dma_start(out=idx_sb[:], in_=idx_t[:])
            dstrel_sb = cpool.tile([CHUNK, T], BF16, tag="dstrel")
            nc.sync.dma_start(out=dstrel_sb[:], in_=dstrel_t[:])
            dinvw_sb = cpool.tile([WINDOW, w_cnt], F32, tag="dinvw")
            nc.sync.dma_start(out=dinvw_sb[:], in_=dinvw_t[:])
            d2_sb = cpool.tile([WINDOW, w_cnt], F32, tag="dinv2")
            nc.vector.tensor_tensor(
                out=d2_sb[:], in0=dinvw_sb[:], in1=dinvw_sb[:],
                op=mybir.AluOpType.mult,
            )

            zrow = cpool.tile([1, d_rep], BF16, tag="zrow")
            nc.vector.memset(zrow[:], 0.0)
            nc.sync.dma_start(out=h2tab[0:1, :], in_=zrow[:1, :])
            nc.sync.dma_start(out=h2tab[N + 1 : N + 2, :], in_=zrow[:1, :])

            def build_s(pool, t0, n, nm):
                """one-hot S for chunks [t0, t0+n) in one DVE op."""
                s_tile = pool.tile([CHUNK, SBATCH * WINDOW], BF16, tag="s", name=nm)
                rel_b = (
                    dstrel_sb[:, t0 : t0 + n]
                    .rearrange("p (b one) -> p b one", one=1)
                    .to_broadcast([CHUNK, n, WINDOW])
                )
                io_v = iota_sb[:, : n * WINDOW].rearrange("p (b j) -> p b j", j=WINDOW)
                s_v = s_tile[:, : n * WINDOW].rearrange("p (b j) -> p b j", j=WINDOW)
                nc.vector.tensor_tensor(
                    out=s_v, in0=io_v, in1=rel_b, op=mybir.AluOpType.is_equal
                )
                return s_tile

            # =========================== PHASE A ===========================
            psum_of_win = {}
            for sec, t0, n, qn in groups:
                gb = gpool.tile([CHUNK, GSZ, d_in], dt_gat, tag="g", name="gb")
                nc.gpsimd.dma_gather(
                    gb[:, :n, :],
                    tab_view(xtab)[sec],
                    idx_sb[:, t0 * 8 : (t0 + n) * 8],
                    n * CHUNK,
                    n * CHUNK,
                    d_in,
                    single_packet=True,
                    queue_num=qn,
                )
                for bt0 in range(t0, t0 + n, SBATCH):
                    bn = min(SBATCH, t0 + n - bt0)
                    s_tile = build_s(spool, bt0, bn, "sA")
                    for t in range(bt0, bt0 + bn):
                        j = t - bt0
                        w, first, last, _sec = chunk_win[t]
                        if first:
                            psum_of_win[w] = psA.tile(
                                [d_in, WINDOW], F32, tag="agg", name="aggps"
                            )
                            xw = selfpool.tile([WINDOW, d_in], dt_gat, tag="xw")
                            nc.sync.dma_start(
                                out=xw[:],
                                in_=xself[w * WINDOW : (w + 1) * WINDOW, :],
                            )
                            nc.tensor.matmul(
                                out=psum_of_win[w][:],
                                lhsT=xw[:],
                                rhs=idb_sb[:],
                                start=True,
                                stop=False,
                            )
                        nc.tensor.matmul(
                            out=psum_of_win[w][:],
                            lhsT=gb[:, t - t0, :],
                            rhs=s_tile[:, j * WINDOW : (j + 1) * WINDOW],
                            start=False,
                            stop=last,
                        )
                        if not last:
                            continue
                        ps = psum_of_win.pop(w)
                        aggw = wpool.tile([d_in, WINDOW], F32, tag="aggw")
                        nc.scalar.activation(out=aggw[:], in_=ps[:], func=Copy)
                        _window_epilogue_A(
                            nc, w, aggw[:], wpool, psW, w1_sb, w2_sb, b1_sb,
                            dinvw_sb, d2_sb, id_sb, idb_sb, h2loc, n_local,
                            d_in, d_hid, n_cls, d_rep, bias_zero,
                        )
                if t0 + n in t_agsplit:
                    # a window range just closed: exchange its h2 rows now,
                    # overlapping the collective with the remaining gathers
                    emit_exchange(t_agsplit.index(t0 + n))

            # ==== h2 exchange, last piece (earlier pieces were emitted ====
            # ==== mid-phase-A and hide under the remaining gathers) ====
            emit_exchange(len(regions) - 1)

            # =========================== PHASE B ===========================
            psum_of_win = {}
            for sec, t0, n, qn in groups:
                g2 = g2pool.tile([CHUNK, GSZ, d_rep], BF16, tag="g2", name="g2b")
                nc.gpsimd.dma_gather(
                    g2[:, :n, :],
                    tab_view(h2tab)[sec],
                    idx_sb[:, t0 * 8 : (t0 + n) * 8],
                    n * CHUNK,
                    n * CHUNK,
                    d_rep,
                    single_packet=True,
                    queue_num=qn,
                )
                for bt0 in range(t0, t0 + n, SBATCH):
                    bn = min(SBATCH, t0 + n - bt0)
                    s_tile = build_s(s2pool, bt0, bn, "sB")
                    for t in range(bt0, bt0 + bn):
                        j = t - bt0
                        w, first, last, _sec = chunk_win[t]
                        if first:
                            psum_of_win[w] = psA.tile(
                                [WINDOW, n_cls], F32, tag="agg", name="agg2ps"
                            )
                            h2w = selfpool.tile([WINDOW, d_rep], BF16, tag="h2w")
                            nc.sync.dma_start(
                                out=h2w[:],
                                in_=h2loc[w * WINDOW : (w + 1) * WINDOW, :],
                            )
                            nc.tensor.matmul(
                                out=psum_of_win[w][:],
                                lhsT=idb_sb[:],
                                rhs=h2w[:, :n_cls],
                                start=True,
                                stop=False,
                            )
                        nc.tensor.matmul(
                            out=psum_of_win[w][:],
                            lhsT=s_tile[:, j * WINDOW : (j + 1) * WINDOW],
                            rhs=g2[:, t - t0, :n_cls],
                            start=False,
                            stop=last,
                        )
                        if not last:
                            continue
                        ps = psum_of_win.pop(w)
                        ob2 = wpool.tile([WINDOW, n_cls], F32, tag="ob2")
                        nc.vector.tensor_scalar(
                            out=ob2[:],
                            in0=ps[:],
                            scalar1=dinvw_sb[:, w : w + 1],
                            scalar2=None,
                            op0=mybir.AluOpType.mult,
                        )
                        ob3 = ob2
                        if not bias2_zero:
                            ob3 = wpool.tile([WINDOW, n_cls], F32, tag="ob3")
                            nc.vector.tensor_tensor(
                                out=ob3[:], in0=ob2[:], in1=b2_sb[:],
                                op=mybir.AluOpType.add,
                            )
                        nrows = min(WINDOW, n_local - w * WINDOW)
                        nc.sync.dma_start(
                            out=out_t[w * WINDOW : w * WINDOW + nrows, :],
                            in_=ob3[:nrows, :],
                        )

    nc.compile()
    return nc


def _window_epilogue_A(
    nc, w, aggT, wpool, psW, w1_sb, w2_sb, b1_sb, dinvw_sb, d2_sb, id_sb,
    idb_sb, h2loc, n_local, d_in, d_hid, n_cls, d_rep, bias_zero,
):
    """aggT [d_in, WINDOW] in SBUF -> replicated h2 rows in DRAM."""
    Relu = mybir.ActivationFunctionType.Relu
    Copy = mybir.ActivationFunctionType.Copy

    # h1 [dst, hid] = aggT.T @ W1
    h1_ps = psW.tile([WINDOW, d_hid], F32, tag="wps", name="h1_ps")
    nc.tensor.matmul(out=h1_ps[:], lhsT=aggT, rhs=w1_sb[:], start=True, stop=True)
    if bias_zero:
        # b1 == 0: relu(dinv*x) = dinv*relu(x); relu straight off PSUM and
        # fold both dinv factors into the final replicate-scale (dinv^2).
        r3_sb = wpool.tile([WINDOW, d_hid], F32, tag="r3")
        nc.scalar.activation(out=r3_sb[:], in_=h1_ps[:], func=Relu)
        fin_scale = d2_sb
    else:
        r_sb = wpool.tile([WINDOW, d_hid], F32, tag="r")
        nc.vector.tensor_scalar(
            out=r_sb[:],
            in0=h1_ps[:],
            scalar1=dinvw_sb[:, w : w + 1],
            scalar2=None,
            op0=mybir.AluOpType.mult,
        )
        r2_sb = wpool.tile([WINDOW, d_hid], F32, tag="r2")
        nc.vector.tensor_tensor(
            out=r2_sb[:], in0=r_sb[:], in1=b1_sb[:], op=mybir.AluOpType.add
        )
        r3_sb = wpool.tile([WINDOW, d_hid], F32, tag="r3")
        nc.scalar.activation(out=r3_sb[:], in_=r2_sb[:], func=Relu)
        fin_scale = dinvw_sb
    # transpose -> [hid, dst]
    rT_ps = psW.tile([d_hid, WINDOW], F32, tag="wps", name="rT_ps")
    nc.tensor.transpose(out=rT_ps[:], in_=r3_sb[:], identity=id_sb[:])
    rT_sb = wpool.tile([d_hid, WINDOW], F32, tag="rTs")
    nc.scalar.activation(out=rT_sb[:], in_=rT_ps[:], func=Copy)
    # h2 [dst, n_cls] = rT.T @ W2; scale; replicate REP x
    h2_ps = psW.tile([WINDOW, n_cls], F32, tag="wps", name="h2_ps")
    nc.tensor.matmul(out=h2_ps[:], lhsT=rT_sb[:], rhs=w2_sb[:], start=True, stop=True)
    h2_sb = wpool.tile([WINDOW, d_rep], BF16, tag="h2s")
    nc.vector.tensor_scalar(
        out=h2_sb[:].rearrange("p (r c) -> p r c", c=n_cls),
        in0=h2_ps[:]
        .rearrange("p (one c) -> p one c", one=1)
        .to_broadcast([WINDOW, REP, n_cls]),
        scalar1=fin_scale[:, w : w + 1],
        scalar2=None,
        op0=mybir.AluOpType.mult,
    )
    nrows = min(WINDOW, n_local - w * WINDOW)
    nc.sync.dma_start(
        out=h2loc[w * WINDOW : w * WINDOW + nrows, :], in_=h2_sb[:nrows, :]
    )


# --------------------------------------------------------------------------
# Entry point
# --------------------------------------------------------------------------
def _make_inputs(x, W1, b1, W2, b2, pp, dt_np, n_cores):
    N, d_in = x.shape
    NP = n_cores * pp["n_local"]
    W1 = np.asarray(W1, np.float32)
    b1 = np.asarray(b1, np.float32)
    W2 = np.asarray(W2, np.float32)
    b2 = np.asarray(b2, np.float32)
    d_hid = W1.shape[1]
    n_cls = W2.shape[1]
    xp = x * pp["dinv"][:, None]
    xtab = np.zeros((NP + 2, d_in), np.float32)
    xtab[pp["tabrow_of_pos"]] = xp
    xtab = xtab.astype(dt_np)
    iota_arr = np.broadcast_to(
        np.tile(np.arange(WINDOW, dtype=np.float32), SBATCH),
        (CHUNK, SBATCH * WINDOW),
    ).astype(np.dtype("bfloat16"))
    shared = {
        "xtab": xtab,
        "w1": W1,
        "w2": W2,
        "b1bc": np.broadcast_to(b1, (WINDOW, d_hid)).astype(np.float32).copy(),
        "b2bc": np.broadcast_to(b2, (WINDOW, n_cls)).astype(np.float32).copy(),
        "iota": iota_arr,
        "ident": np.eye(WINDOW, dtype=np.float32),
        "identb": np.eye(WINDOW, dtype=np.float32).astype(dt_np),
    }
    n_local = pp["n_local"]
    in_maps = []
    for c, pc in enumerate(pp["per_core"]):
        m = dict(shared)
        xs = np.zeros((n_local, d_in), np.float32)
        sl = pp["inv"][c * n_local : (c + 1) * n_local]
        xs[sl >= 0] = xp[sl[sl >= 0]]
        m["xself"] = xs.astype(dt_np)
        m["idx16"] = pc["idx16"]
        m["dstrel"] = pc["dstrel"].astype(np.dtype("bfloat16"))
        m["dinvw"] = pc["dinvw"]
        in_maps.append(m)
    return in_maps


def _run(x, edge_index, W1, b1, W2, b2, n_cores, trace=False):
    x = np.asarray(x, dtype=np.float32)
    N, d_in = x.shape
    d_hid = np.asarray(W1).shape[1]
    n_cls = np.asarray(W2).shape[1]
    assert d_in == 128 and d_hid == 128

    pp = _preprocess(x, edge_index, n_cores)
    dt_gat = BF16 if GATHER_BF16 else F32
    np_gat = np.dtype("bfloat16") if GATHER_BF16 else np.dtype("float32")

    nc = bacc.Bacc("TRN2", target_bir_lowering=False, debug=False, num_swdge_queues=4)
    _build(
        nc,
        N=n_cores * pp["n_local"],
        n_local=pp["n_local"],
        d_in=d_in,
        d_hid=d_hid,
        n_cls=n_cls,
        pp=pp,
        n_cores=n_cores,
        dt_gat=dt_gat,
        bias_zero=bool(np.all(np.asarray(b1) == 0)),
        bias2_zero=bool(np.all(np.asarray(b2) == 0)),
    )

    import ml_dtypes  # noqa

    in_maps = _make_inputs(x, W1, b1, W2, b2, pp, np_gat, n_cores)
    res = run_bass_kernel_spmd(nc, in_maps, list(range(n_cores)), trace=trace)
    outs = [res.results[c]["out"] for c in range(n_cores)]
    full = np.concatenate(outs, axis=0)[pp["permpos"]]
    return full.astype(np.float32), res


def kernel(x, edge_index, W1, b1, W2, b2):
    out, _ = _run(x, edge_index, W1, b1, W2, b2, N_CORES)
    return out



# revision 31
# speedup vs baseline: 1.0090x; 1.0090x over previous
"""GCN 2-layer (PyG GCNConv x2 + ReLU) Bass kernel for Trainium2, 8-core SPMD.

Strategy:
  - Host: compute symmetric normalization dinv = deg^-1/2 (self-loop incl.),
    fold dinv[src] into a prescaled gather table (x * dinv).  Dst nodes are
    RELABELED via degree-balanced LPT binning into 8 cores x 49 windows of
    128 nodes, so every (core, window) has a near-equal edge count and chunk
    padding is <1%.  Per core, edges sorted by dst window and packed into
    128-edge chunks (one matmul each).  dma_gather uses int16 indices, so
    the node table is addressed via two 32768-row views (LOW/HIGH); srcs in
    the overlap may go to either section and are assigned to fill chunk
    boundaries exactly.  Each window's LOW and HIGH chunks run back to
    back so its PSUM accumulation closes in one pass (single eviction,
    epilogues spread across the phase).  Biases are folded away when zero
    (relu(dinv*x) = dinv*relu(x); dinv^2 in the final replicate-scale).
    Self-loop edges are NOT gathered: the permuted
    table makes each window's own rows contiguous, so they stream in with a
    plain DMA and join the accumulation as one identity-operand matmul.
  - Device per core:
      Gathers are issued round-robin on the 4 SWDGE queues (num_swdge_
      queues=4).  Descriptor generation for queues 1-3 runs on their own Q7
      core pairs concurrently under queue 0's engine-blocking gather,
      cutting effective desc-gen from 8.4 ns/idx to ~2.6 ns/idx.
      Phase A (layer 1): dma_gather source rows of the prescaled x-table ->
        G [128e, d_in]; build one-hot S [128e, 128dst] on DVE (bf16 iota ==
        dst_rel); PE matmul accumulates G.T @ S into PSUM [d_in, 128dst]
        per window.  Per window: x W1 (PE), scale by dinv[dst], +b1, ReLU;
        transpose (PE); x W2; scale by dinv[dst]; replicate 64x -> 256B
        rows of the h2 table, DMA out.
      AllGather h2 shards -> full replicated table.
      Phase B (layer 2): same chunk structure; gather h2 rows, matmul
        S.T @ G2[:, :2] accumulated per window; scale by dinv[dst], +b2.
    Output rows are in permuted order; the host scatters them back.
"""

import numpy as np

import concourse.bass as bass
import concourse.mybir as mybir
import concourse.tile as tile
from concourse import bacc
from concourse.bass_utils import run_bass_kernel_spmd

F32 = mybir.dt.float32
BF16 = mybir.dt.bfloat16
I16 = mybir.dt.int16

N_CORES = 8
WINDOW = 128  # dst nodes per PSUM accumulation window
CHUNK = 128  # edges per matmul chunk
GSZ = 8  # max chunks per dma_gather instruction (1024 idxs, single-packet)
SBATCH = 8  # chunks per S-build DVE op
HALF = 32768  # int16 index range
REP = 64  # h2 replication (64x2 bf16 cols -> 256B rows)
W_SPLITS = (0, 25, 49)  # h2 AllGather pieces: window ranges, exchanged as each closes
GATHER_BF16 = True  # layer-1 gather table + chunk matmuls in bf16


# --------------------------------------------------------------------------
# Host preprocessing
# --------------------------------------------------------------------------
def _preprocess(x, edge_index, n_cores):
    N = x.shape[0]
    src = np.asarray(edge_index[0], dtype=np.int64)
    dst = np.asarray(edge_index[1], dtype=np.int64)
    # degree includes the self-loop (handled separately on-device)
    deg = (np.bincount(dst, minlength=N) + 1).astype(np.float64)
    dinv = np.where(deg > 0, 1.0 / np.sqrt(deg), 0.0).astype(np.float32)

    # ---- degree-balanced dst binning: 392 bins (8 cores x 49 windows), ----
    # ---- capacity 128 nodes each; LPT greedy on in-degree ----
    import heapq

    w_cnt = 49
    n_bins = n_cores * w_cnt
    n_local = w_cnt * WINDOW  # 6272 (padded shard; empty slots get dinv 0)
    deg_ns = np.bincount(dst, minlength=N).astype(np.float64)
    order_nodes = np.argsort(-deg_ns, kind="stable")
    heap = [(0.0, b, 0) for b in range(n_bins)]  # (edge_sum, bin, n_nodes)
    heapq.heapify(heap)
    bin_of = np.empty(N, dtype=np.int64)
    slot_of = np.empty(N, dtype=np.int64)
    stash = []
    for n in order_nodes:
        while True:
            s_, b, cnt = heapq.heappop(heap)
            if cnt < WINDOW:
                break
            stash.append((s_, b, cnt))  # full bin, drop
        bin_of[n] = b
        slot_of[n] = cnt
        heapq.heappush(heap, (s_ + deg_ns[n], b, cnt + 1))
    # permuted position of node n
    permpos = bin_of * WINDOW + slot_of  # in [0, 50176)
    # inverse: node at permuted position p (or -1)
    inv = np.full(n_cores * n_local, -1, dtype=np.int64)
    inv[permpos] = np.arange(N)

    # table rows: 0 = zero, 1..NP = permuted slots, NP+1 = zero.
    # row(node n) = permpos[n] + 1
    # LOW view = rows [0, min(HALF, NP+2)); HIGH view = rows [HB, HB+HALF)
    NP = n_cores * n_local
    HB = max(0, NP + 2 - HALF)
    # section 0 restricted to the first AllGather region so phase B's
    # first pass can gather while the second exchange piece is in flight
    lowmax_row = min(HALF, NP + 2, 1 + n_cores * W_SPLITS[1] * WINDOW)
    pad_low = 0
    pad_high = NP + 1 - HB

    # table layout: row 0 = zero; A-region rows [1, 1+nc*ASZ) hold the
    # first A_WIN windows of every shard (early AllGather); B-region rows
    # hold the rest; last row = zero.  rowof(p) maps permuted position.
    p_c = permpos // n_local
    p_off = permpos % n_local
    tabrow_of_pos = np.zeros(N, dtype=np.int64)
    roff = 1
    for r in range(len(W_SPLITS) - 1):
        lo, hi = W_SPLITS[r] * WINDOW, W_SPLITS[r + 1] * WINDOW
        rsz = hi - lo
        m = (p_off >= lo) & (p_off < hi)
        tabrow_of_pos[m] = roff + p_c[m] * rsz + (p_off[m] - lo)
        roff += n_cores * rsz
    rows_all = tabrow_of_pos[src]  # gather row per edge
    dstbin = bin_of[dst]
    dstslot = slot_of[dst]

    # group edges by dst bin
    eorder = np.argsort(dstbin, kind="stable")
    g_rows = rows_all[eorder]
    g_slot = dstslot[eorder]
    g_bin = dstbin[eorder]
    starts = np.searchsorted(g_bin, np.arange(n_bins + 1))

    # per (core, window): split edges into LOW/HIGH with flex edges
    # (rows in [HB, lowmax) fit either view). kw must be uniform across
    # cores, so pick per-window section targets jointly: kw_lo = max_c
    # ceil(lo_only/128), all cores fill LOW to that boundary with flex.
    split = {}
    for b in range(n_bins):
        c, w = divmod(b, w_cnt)
        rows = g_rows[starts[b] : starts[b + 1]]
        rel = g_slot[starts[b] : starts[b + 1]].astype(np.float32)
        is_lo_only = rows < HB
        is_hi_only = rows >= lowmax_row
        is_flex = ~is_lo_only & ~is_hi_only
        split[(c, w)] = (
            (rows[is_lo_only], rel[is_lo_only]),
            (rows[is_hi_only], rel[is_hi_only]),
            (rows[is_flex], rel[is_flex]),
        )
    parts = {}
    kw_lo = np.zeros(w_cnt, dtype=np.int64)
    kw_hi = np.zeros(w_cnt, dtype=np.int64)
    for w in range(w_cnt):
        klo_a = max(
            1, max(-(-len(split[(c, w)][0][0]) // CHUNK) for c in range(n_cores))
        )
        khi_a = max(
            -(
                -(len(split[(c, w)][1][0]) + max(
                    0,
                    len(split[(c, w)][2][0])
                    - (klo_a * CHUNK - len(split[(c, w)][0][0])),
                ))
                // CHUNK
            )
            for c in range(n_cores)
        )
        khi_a = max(1, khi_a)
        # alternative: minimize HIGH first
        khi_b = max(
            1, max(-(-len(split[(c, w)][1][0]) // CHUNK) for c in range(n_cores))
        )
        klo_b = max(
            -(
                -(len(split[(c, w)][0][0]) + max(
                    0,
                    len(split[(c, w)][2][0])
                    - (khi_b * CHUNK - len(split[(c, w)][1][0])),
                ))
                // CHUNK
            )
            for c in range(n_cores)
        )
        klo_b = max(1, klo_b)
        if klo_a + khi_a <= klo_b + khi_b:
            kw_lo[w], kw_hi[w], fill_low = klo_a, khi_a, True
        else:
            kw_lo[w], kw_hi[w], fill_low = klo_b, khi_b, False
        for c in range(n_cores):
            (lo_r, lo_s), (hi_r, hi_s), (fx_r, fx_s) = split[(c, w)]
            if fill_low:
                take = min(len(fx_r), kw_lo[w] * CHUNK - len(lo_r))
            else:
                take = len(fx_r) - min(len(fx_r), kw_hi[w] * CHUNK - len(hi_r))
            parts[(c, w, 0)] = (
                np.concatenate([lo_r, fx_r[:take]]),
                np.concatenate([lo_s, fx_s[:take]]),
            )
            parts[(c, w, 1)] = (
                np.concatenate([hi_r, fx_r[take:]]) - HB,
                np.concatenate([hi_s, fx_s[take:]]),
            )
    T_lo, T_hi = int(kw_lo.sum()), int(kw_hi.sum())
    T = T_lo + T_hi

    # chunk order: per window, LOW chunks then HIGH chunks (PSUM stays
    # open across both sections; one eviction per window).
    chunk_win = []  # (window, first_overall, last_overall, section)
    for w in range(w_cnt):
        kt = int(kw_lo[w] + kw_hi[w])
        k = 0
        for sec, kws in ((0, kw_lo), (1, kw_hi)):
            for _ in range(int(kws[w])):
                chunk_win.append((w, k == 0, k == kt - 1, sec))
                k += 1

    per_core = []
    for c in range(n_cores):
        idx_lin = np.zeros(T * CHUNK, dtype=np.int32)
        dstrel = np.zeros((CHUNK, T), dtype=np.float32)
        t = 0
        for w in range(w_cnt):
            for sec, kws, padrow in ((0, kw_lo, pad_low), (1, kw_hi, pad_high)):
                rows, rel = parts[(c, w, sec)]
                n_e = len(rows)
                n_slots = int(kws[w]) * CHUNK
                buf = np.full(n_slots, padrow, dtype=np.int32)
                buf[:n_e] = rows
                idx_lin[t * CHUNK : t * CHUNK + n_slots] = buf
                rbuf = np.zeros(n_slots, dtype=np.float32)
                rbuf[:n_e] = rel
                dstrel[:, t : t + int(kws[w])] = rbuf.reshape(int(kws[w]), CHUNK).T
                t += int(kws[w])
        assert t == T
        idx16 = idx_lin.astype(np.int16).reshape(T * CHUNK // 16, 16).T  # [16, S]
        idx16 = np.tile(idx16, (8, 1))  # [128, S]

        dinvw = np.zeros((WINDOW, w_cnt), dtype=np.float32)
        for w in range(w_cnt):
            nodes = inv[(c * w_cnt + w) * WINDOW : (c * w_cnt + w + 1) * WINDOW]
            valid = nodes >= 0
            dinvw[valid, w] = dinv[nodes[valid]]
        per_core.append({"idx16": idx16, "dstrel": dstrel, "dinvw": dinvw})

    kw_tot = kw_lo + kw_hi
    t_agsplit = [int(kw_tot[: W_SPLITS[r + 1]].sum()) for r in range(len(W_SPLITS) - 2)]

    return {
        "t_agsplit": t_agsplit,
        "tabrow_of_pos": tabrow_of_pos,
        "n_local": n_local,
        "w_cnt": w_cnt,
        "kw_lo": kw_lo,
        "kw_hi": kw_hi,
        "T_lo": T_lo,
        "T_hi": T_hi,
        "T": T,
        "HB": HB,
        "chunk_win": chunk_win,
        "dinv": dinv,
        "permpos": permpos,
        "inv": inv,
        "per_core": per_core,
    }


# --------------------------------------------------------------------------
# Device kernel builder (one program, SPMD across cores)
# --------------------------------------------------------------------------
def _build(nc, *, N, n_local, d_in, d_hid, n_cls, pp, n_cores, dt_gat, bias_zero, bias2_zero):
    Relu = mybir.ActivationFunctionType.Relu
    Copy = mybir.ActivationFunctionType.Copy
    T, T_lo = pp["T"], pp["T_lo"]
    w_cnt, HB = pp["w_cnt"], pp["HB"]
    chunk_win = pp["chunk_win"]
    d_rep = REP * n_cls  # 64 cols of f32 -> 256B rows

    xtab = nc.dram_tensor("xtab", [N + 2, d_in], dt_gat, kind="ExternalInput")
    w1 = nc.dram_tensor("w1", [d_in, d_hid], F32, kind="ExternalInput")
    w2 = nc.dram_tensor("w2", [d_hid, n_cls], F32, kind="ExternalInput")
    b1bc = nc.dram_tensor("b1bc", [WINDOW, d_hid], F32, kind="ExternalInput")
    b2bc = nc.dram_tensor("b2bc", [WINDOW, n_cls], F32, kind="ExternalInput")
    iota = nc.dram_tensor("iota", [CHUNK, SBATCH * WINDOW], BF16, kind="ExternalInput")
    ident = nc.dram_tensor("ident", [WINDOW, WINDOW], F32, kind="ExternalInput")
    identb = nc.dram_tensor("identb", [WINDOW, WINDOW], dt_gat, kind="ExternalInput")
    xself = nc.dram_tensor("xself", [n_local, d_in], dt_gat, kind="ExternalInput")
    idx_t = nc.dram_tensor("idx16", [CHUNK, T * 8], I16, kind="ExternalInput")
    dstrel_t = nc.dram_tensor("dstrel", [CHUNK, T], BF16, kind="ExternalInput")
    dinvw_t = nc.dram_tensor("dinvw", [WINDOW, w_cnt], F32, kind="ExternalInput")
    out_t = nc.dram_tensor("out", [n_local, n_cls], F32, kind="ExternalOutput")

    h2loc = nc.dram_tensor("h2loc", [n_local, d_rep], BF16)
    h2tab = nc.dram_tensor("h2tab", [N + 2, d_rep], BF16, addr_space="Shared")

    # gather groups: runs of chunks sharing (window, section), split into
    # near-equal pieces of <= GSZ; round-robin across the 4 SWDGE queues.
    groups = []
    qn = 0
    t0 = 0
    while t0 < T:
        w0, _, _, sec0 = chunk_win[t0]
        t1 = t0
        while t1 < T and chunk_win[t1][0] == w0 and chunk_win[t1][3] == sec0:
            t1 += 1
        run = t1 - t0
        n_pieces = -(-run // GSZ)
        base = run // n_pieces
        rem = run % n_pieces
        tt0 = t0
        for i in range(n_pieces):
            n = base + (1 if i < rem else 0)
            groups.append((sec0, tt0, n, qn % 4))
            qn += 1
            tt0 += n
        t0 = t1

    # (row offset into h2tab/xtab, local row range) per AllGather piece
    regions = []
    roff = 1
    for r in range(len(W_SPLITS) - 1):
        lo, hi = W_SPLITS[r] * WINDOW, W_SPLITS[r + 1] * WINDOW
        regions.append((roff, lo, hi))
        roff += n_cores * (hi - lo)
    t_agsplit = pp["t_agsplit"]

    def emit_exchange(r):
        roff_r, lo, hi = regions[r]
        if n_cores > 1:
            nc.gpsimd.collective_compute(
                "AllGather",
                mybir.AluOpType.bypass,
                replica_groups=[list(range(n_cores))],
                ins=[h2loc[lo:hi, :]],
                outs=[h2tab[roff_r : roff_r + n_cores * (hi - lo), :]],
            )
        else:
            nc.sync.dma_start(
                out=h2tab[roff_r : roff_r + (hi - lo), :], in_=h2loc[lo:hi, :]
            )

    def tab_view(tab):
        return [
            tab[0 : min(HALF, N + 2), :],
            tab[HB : min(HB + HALF, N + 2), :],
        ]

    with tile.TileContext(nc) as tc:
        with (
            tc.tile_pool(name="const", bufs=1) as cpool,
            tc.tile_pool(name="gbuf", bufs=12) as gpool,
            tc.tile_pool(name="g2buf", bufs=12) as g2pool,
            tc.tile_pool(name="sbat", bufs=8) as spool,
            tc.tile_pool(name="sbat2", bufs=8) as s2pool,
            tc.tile_pool(name="wtmp", bufs=3) as wpool,
            tc.tile_pool(name="selfp", bufs=3) as selfpool,
            tc.tile_pool(name="psA", bufs=4, space="PSUM") as psA,
            tc.tile_pool(name="psW", bufs=4, space="PSUM") as psW,
        ):
            # ---- constants into SBUF ----
            w1_sb = cpool.tile([d_in, d_hid], F32, tag="w1")
            nc.sync.dma_start(out=w1_sb[:], in_=w1[:])
            w2_sb = cpool.tile([d_hid, n_cls], F32, tag="w2")
            nc.sync.dma_start(out=w2_sb[:], in_=w2[:])
            b1_sb = cpool.tile([WINDOW, d_hid], F32, tag="b1")
            nc.sync.dma_start(out=b1_sb[:], in_=b1bc[:])
            b2_sb = cpool.tile([WINDOW, n_cls], F32, tag="b2")
            nc.sync.dma_start(out=b2_sb[:], in_=b2bc[:])
            iota_sb = cpool.tile([CHUNK, SBATCH * WINDOW], BF16, tag="iota")
            nc.sync.dma_start(out=iota_sb[:], in_=iota[:])
            id_sb = cpool.tile([WINDOW, WINDOW], F32, tag="ident")
            nc.sync.dma_start(out=id_sb[:], in_=ident[:])
            idb_sb = cpool.tile([WINDOW, WINDOW], dt_gat, tag="identb")
            nc.sync.dma_start(out=idb_sb[:], in_=identb[:])
            idx_sb = cpool.tile([CHUNK, T * 8], I16, tag="idx")
            nc.sync.dma_start(out=idx_sb[:], in_=idx_t[:])
            dstrel_sb = cpool.tile([CHUNK, T], BF16, tag="dstrel")
            nc.sync.dma_start(out=dstrel_sb[:], in_=dstrel_t[:])
            dinvw_sb = cpool.tile([WINDOW, w_cnt], F32, tag="dinvw")
            nc.sync.dma_start(out=dinvw_sb[:], in_=dinvw_t[:])
            d2_sb = cpool.tile([WINDOW, w_cnt], F32, tag="dinv2")
            nc.vector.tensor_tensor(
                out=d2_sb[:], in0=dinvw_sb[:], in1=dinvw_sb[:],
                op=mybir.AluOpType.mult,
            )

            zrow = cpool.tile([1, d_rep], BF16, tag="zrow")
            nc.vector.memset(zrow[:], 0.0)
            nc.sync.dma_start(out=h2tab[0:1, :], in_=zrow[:1, :])
            nc.sync.dma_start(out=h2tab[N + 1 : N + 2, :], in_=zrow[:1, :])

            def build_s(pool, t0, n, nm):
                """one-hot S for chunks [t0, t0+n) in one DVE op."""
                s_tile = pool.tile([CHUNK, SBATCH * WINDOW], BF16, tag="s", name=nm)
                rel_b = (
                    dstrel_sb[:, t0 : t0 + n]
                    .rearrange("p (b one) -> p b one", one=1)
                    .to_broadcast([CHUNK, n, WINDOW])
                )
                io_v = iota_sb[:, : n * WINDOW].rearrange("p (b j) -> p b j", j=WINDOW)
                s_v = s_tile[:, : n * WINDOW].rearrange("p (b j) -> p b j", j=WINDOW)
                nc.vector.tensor_tensor(
                    out=s_v, in0=io_v, in1=rel_b, op=mybir.AluOpType.is_equal
                )
                return s_tile

            # =========================== PHASE A ===========================
            psum_of_win = {}
            for sec, t0, n, qn in groups:
                gb = gpool.tile([CHUNK, GSZ, d_in], dt_gat, tag="g", name="gb")
                nc.gpsimd.dma_gather(
                    gb[:, :n, :],
                    tab_view(xtab)[sec],
                    idx_sb[:, t0 * 8 : (t0 + n) * 8],
                    n * CHUNK,
                    n * CHUNK,
                    d_in,
                    single_packet=True,
                    queue_num=qn,
                )
                for bt0 in range(t0, t0 + n, SBATCH):
                    bn = min(SBATCH, t0 + n - bt0)
                    s_tile = build_s(spool, bt0, bn, "sA")
                    for t in range(bt0, bt0 + bn):
                        j = t - bt0
                        w, first, last, _sec = chunk_win[t]
                        if first:
                            psum_of_win[w] = psA.tile(
                                [d_in, WINDOW], F32, tag="agg", name="aggps"
                            )
                            xw = selfpool.tile([WINDOW, d_in], dt_gat, tag="xw")
                            nc.sync.dma_start(
                                out=xw[:],
                                in_=xself[w * WINDOW : (w + 1) * WINDOW, :],
                            )
                            nc.tensor.matmul(
                                out=psum_of_win[w][:],
                                lhsT=xw[:],
                                rhs=idb_sb[:],
                                start=True,
                                stop=False,
                            )
                        nc.tensor.matmul(
                            out=psum_of_win[w][:],
                            lhsT=gb[:, t - t0, :],
                            rhs=s_tile[:, j * WINDOW : (j + 1) * WINDOW],
                            start=False,
                            stop=last,
                        )
                        if not last:
                            continue
                        ps = psum_of_win.pop(w)
                        aggw = wpool.tile([d_in, WINDOW], F32, tag="aggw")
                        nc.scalar.activation(out=aggw[:], in_=ps[:], func=Copy)
                        _window_epilogue_A(
                            nc, w, aggw[:], wpool, psW, w1_sb, w2_sb, b1_sb,
                            dinvw_sb, d2_sb, id_sb, idb_sb, h2loc, n_local,
                            d_in, d_hid, n_cls, d_rep, bias_zero,
                        )
                if t0 + n in t_agsplit:
                    # a window range just closed: exchange its h2 rows now,
                    # overlapping the collective with the remaining gathers
                    emit_exchange(t_agsplit.index(t0 + n))

            # ==== h2 exchange, last piece (earlier pieces were emitted ====
            # ==== mid-phase-A and hide under the remaining gathers) ====
            emit_exchange(len(regions) - 1)

            # =========================== PHASE B ===========================
            # Section-major: pass 0 gathers only rows < AVIEW (the region
            # exchanged by the FIRST AllGather piece), so its gathers flow
            # straight out of phase A while the second piece is in flight;
            # pass 1 (HIGH view) accumulates on top of the pass-0 partials.
            kw_lo_a = pp["kw_lo"]
            kw_hi_a = pp["kw_hi"]
            chunk_base = [0] * (w_cnt + 1)
            for w in range(w_cnt):
                chunk_base[w + 1] = chunk_base[w] + int(kw_lo_a[w] + kw_hi_a[w])
            AVIEW = 1 + n_cores * W_SPLITS[1] * WINDOW
            out2_sb = cpool.tile([WINDOW, w_cnt * n_cls], F32, tag="out2")
            bgroups = []
            qnb = 0
            for sec in (0, 1):
                for w in range(w_cnt):
                    t0w = chunk_base[w] + (0 if sec == 0 else int(kw_lo_a[w]))
                    run = int(kw_lo_a[w]) if sec == 0 else int(kw_hi_a[w])
                    n_pieces = -(-run // GSZ)
                    gbase = run // n_pieces
                    grem = run % n_pieces
                    tt0 = t0w
                    for i in range(n_pieces):
                        n = gbase + (1 if i < grem else 0)
                        bgroups.append((sec, w, tt0, n, qnb % 4))
                        qnb += 1
                        tt0 += n
            psum_of_win = {}
            for sec, w, t0, n, qn in bgroups:
                g2 = g2pool.tile([CHUNK, GSZ, d_rep], BF16, tag="g2", name="g2b")
                view = h2tab[0:AVIEW, :] if sec == 0 else tab_view(h2tab)[1]
                nc.gpsimd.dma_gather(
                    g2[:, :n, :],
                    view,
                    idx_sb[:, t0 * 8 : (t0 + n) * 8],
                    n * CHUNK,
                    n * CHUNK,
                    d_rep,
                    single_packet=True,
                    queue_num=qn,
                )
                for bt0 in range(t0, t0 + n, SBATCH):
                    bn = min(SBATCH, t0 + n - bt0)
                    s_tile = build_s(s2pool, bt0, bn, "sB")
                    for t in range(bt0, bt0 + bn):
                        j = t - bt0
                        sec0_first = sec == 0 and t == chunk_base[w]
                        sec0_last = (
                            sec == 0 and t == chunk_base[w] + int(kw_lo_a[w]) - 1
                        )
                        sec1_first = (
                            sec == 1 and t == chunk_base[w] + int(kw_lo_a[w])
                        )
                        sec1_last = sec == 1 and t == chunk_base[w + 1] - 1
                        if sec0_first:
                            psum_of_win[w] = psA.tile(
                                [WINDOW, n_cls], F32, tag="agg", name="agg2ps"
                            )
                            h2w = selfpool.tile([WINDOW, d_rep], BF16, tag="h2w")
                            nc.sync.dma_start(
                                out=h2w[:],
                                in_=h2loc[w * WINDOW : (w + 1) * WINDOW, :],
                            )
                            nc.tensor.matmul(
                                out=psum_of_win[w][:],
                                lhsT=idb_sb[:],
                                rhs=h2w[:, :n_cls],
                                start=True,
                                stop=False,
                            )
                        elif sec1_first:
                            psum_of_win[w] = psA.tile(
                                [WINDOW, n_cls], F32, tag="agg", name="agg2ps"
                            )
                        nc.tensor.matmul(
                            out=psum_of_win[w][:],
                            lhsT=s_tile[:, j * WINDOW : (j + 1) * WINDOW],
                            rhs=g2[:, t - t0, :n_cls],
                            start=sec1_first,
                            stop=sec0_last or sec1_last,
                        )
                        osl = out2_sb[:, w * n_cls : (w + 1) * n_cls]
                        if sec0_last:
                            ps = psum_of_win.pop(w)
                            nc.scalar.activation(out=osl, in_=ps[:], func=Copy)
                        elif sec1_last:
                            ps = psum_of_win.pop(w)
                            ob = wpool.tile([WINDOW, n_cls], F32, tag="ob")
                            nc.vector.tensor_tensor(
                                out=ob[:], in0=ps[:], in1=osl,
                                op=mybir.AluOpType.add,
                            )
                            ob2 = wpool.tile([WINDOW, n_cls], F32, tag="ob2")
                            nc.vector.tensor_scalar(
                                out=ob2[:],
                                in0=ob[:],
                                scalar1=dinvw_sb[:, w : w + 1],
                                scalar2=None,
                                op0=mybir.AluOpType.mult,
                            )
                            ob3 = ob2
                            if not bias2_zero:
                                ob3 = wpool.tile([WINDOW, n_cls], F32, tag="ob3")
                                nc.vector.tensor_tensor(
                                    out=ob3[:], in0=ob2[:], in1=b2_sb[:],
                                    op=mybir.AluOpType.add,
                                )
                            nrows = min(WINDOW, n_local - w * WINDOW)
                            nc.sync.dma_start(
                                out=out_t[w * WINDOW : w * WINDOW + nrows, :],
                                in_=ob3[:nrows, :],
                            )

    nc.compile()
    return nc


def _window_epilogue_A(
    nc, w, aggT, wpool, psW, w1_sb, w2_sb, b1_sb, dinvw_sb, d2_sb, id_sb,
    idb_sb, h2loc, n_local, d_in, d_hid, n_cls, d_rep, bias_zero,
):
    """aggT [d_in, WINDOW] in SBUF -> replicated h2 rows in DRAM."""
    Relu = mybir.ActivationFunctionType.Relu
    Copy = mybir.ActivationFunctionType.Copy

    # h1 [dst, hid] = aggT.T @ W1
    h1_ps = psW.tile([WINDOW, d_hid], F32, tag="wps", name="h1_ps")
    nc.tensor.matmul(out=h1_ps[:], lhsT=aggT, rhs=w1_sb[:], start=True, stop=True)
    if bias_zero:
        # b1 == 0: relu(dinv*x) = dinv*relu(x); relu straight off PSUM and
        # fold both dinv factors into the final replicate-scale (dinv^2).
        r3_sb = wpool.tile([WINDOW, d_hid], F32, tag="r3")
        nc.scalar.activation(out=r3_sb[:], in_=h1_ps[:], func=Relu)
        fin_scale = d2_sb
    else:
        r_sb = wpool.tile([WINDOW, d_hid], F32, tag="r")
        nc.vector.tensor_scalar(
            out=r_sb[:],
            in0=h1_ps[:],
            scalar1=dinvw_sb[:, w : w + 1],
            scalar2=None,
            op0=mybir.AluOpType.mult,
        )
        r2_sb = wpool.tile([WINDOW, d_hid], F32, tag="r2")
        nc.vector.tensor_tensor(
            out=r2_sb[:], in0=r_sb[:], in1=b1_sb[:], op=mybir.AluOpType.add
        )
        r3_sb = wpool.tile([WINDOW, d_hid], F32, tag="r3")
        nc.scalar.activation(out=r3_sb[:], in_=r2_sb[:], func=Relu)
        fin_scale = dinvw_sb
    # transpose -> [hid, dst]
    rT_ps = psW.tile([d_hid, WINDOW], F32, tag="wps", name="rT_ps")
    nc.tensor.transpose(out=rT_ps[:], in_=r3_sb[:], identity=id_sb[:])
    rT_sb = wpool.tile([d_hid, WINDOW], F32, tag="rTs")
    nc.scalar.activation(out=rT_sb[:], in_=rT_ps[:], func=Copy)
    # h2 [dst, n_cls] = rT.T @ W2; scale; replicate REP x
    h2_ps = psW.tile([WINDOW, n_cls], F32, tag="wps", name="h2_ps")
    nc.tensor.matmul(out=h2_ps[:], lhsT=rT_sb[:], rhs=w2_sb[:], start=True, stop=True)
    h2_sb = wpool.tile([WINDOW, d_rep], BF16, tag="h2s")
    nc.vector.tensor_scalar(
        out=h2_sb[:].rearrange("p (r c) -> p r c", c=n_cls),
        in0=h2_ps[:]
        .rearrange("p (one c) -> p one c", one=1)
        .to_broadcast([WINDOW, REP, n_cls]),
        scalar1=fin_scale[:, w : w + 1],
        scalar2=None,
        op0=mybir.AluOpType.mult,
    )
    nrows = min(WINDOW, n_local - w * WINDOW)
    nc.sync.dma_start(
        out=h2loc[w * WINDOW : w * WINDOW + nrows, :], in_=h2_sb[:nrows, :]
    )


# --------------------------------------------------------------------------
# Entry point
# --------------------------------------------------------------------------
def _make_inputs(x, W1, b1, W2, b2, pp, dt_np, n_cores):
    N, d_in = x.shape
    NP = n_cores * pp["n_local"]
    W1 = np.asarray(W1, np.float32)
    b1 = np.asarray(b1, np.float32)
    W2 = np.asarray(W2, np.float32)
    b2 = np.asarray(b2, np.float32)
    d_hid = W1.shape[1]
    n_cls = W2.shape[1]
    xp = x * pp["dinv"][:, None]
    xtab = np.zeros((NP + 2, d_in), np.float32)
    xtab[pp["tabrow_of_pos"]] = xp
    xtab = xtab.astype(dt_np)
    iota_arr = np.broadcast_to(
        np.tile(np.arange(WINDOW, dtype=np.float32), SBATCH),
        (CHUNK, SBATCH * WINDOW),
    ).astype(np.dtype("bfloat16"))
    shared = {
        "xtab": xtab,
        "w1": W1,
        "w2": W2,
        "b1bc": np.broadcast_to(b1, (WINDOW, d_hid)).astype(np.float32).copy(),
        "b2bc": np.broadcast_to(b2, (WINDOW, n_cls)).astype(np.float32).copy(),
        "iota": iota_arr,
        "ident": np.eye(WINDOW, dtype=np.float32),
        "identb": np.eye(WINDOW, dtype=np.float32).astype(dt_np),
    }
    n_local = pp["n_local"]
    in_maps = []
    for c, pc in enumerate(pp["per_core"]):
        m = dict(shared)
        xs = np.zeros((n_local, d_in), np.float32)
        sl = pp["inv"][c * n_local : (c + 1) * n_local]
        xs[sl >= 0] = xp[sl[sl >= 0]]
        m["xself"] = xs.astype(dt_np)
        m["idx16"] = pc["idx16"]
        m["dstrel"] = pc["dstrel"].astype(np.dtype("bfloat16"))
        m["dinvw"] = pc["dinvw"]
        in_maps.append(m)
    return in_maps


def _run(x, edge_index, W1, b1, W2, b2, n_cores, trace=False):
    x = np.asarray(x, dtype=np.float32)
    N, d_in = x.shape
    d_hid = np.asarray(W1).shape[1]
    n_cls = np.asarray(W2).shape[1]
    assert d_in == 128 and d_hid == 128

    pp = _preprocess(x, edge_index, n_cores)
    dt_gat = BF16 if GATHER_BF16 else F32
    np_gat = np.dtype("bfloat16") if GATHER_BF16 else np.dtype("float32")

    nc = bacc.Bacc("TRN2", target_bir_lowering=False, debug=False, num_swdge_queues=4)
    _build(
        nc,
        N=n_cores * pp["n_local"],
        n_local=pp["n_local"],
        d_in=d_in,
        d_hid=d_hid,
        n_cls=n_cls,
        pp=pp,
        n_cores=n_cores,
        dt_gat=dt_gat,
        bias_zero=bool(np.all(np.asarray(b1) == 0)),
        bias2_zero=bool(np.all(np.asarray(b2) == 0)),
    )

    import ml_dtypes  # noqa

    in_maps = _make_inputs(x, W1, b1, W2, b2, pp, np_gat, n_cores)
    res = run_bass_kernel_spmd(nc, in_maps, list(range(n_cores)), trace=trace)
    outs = [res.results[c]["out"] for c in range(n_cores)]
    full = np.concatenate(outs, axis=0)[pp["permpos"]]
    return full.astype(np.float32), res


def kernel(x, edge_index, W1, b1, W2, b2):
    out, _ = _run(x, edge_index, W1, b1, W2, b2, N_CORES)
    return out



# revision 32
# speedup vs baseline: 1.0123x; 1.0032x over previous
"""GCN 2-layer (PyG GCNConv x2 + ReLU) Bass kernel for Trainium2, 8-core SPMD.

Strategy:
  - Host: compute symmetric normalization dinv = deg^-1/2 (self-loop incl.),
    fold dinv[src] into a prescaled gather table (x * dinv).  Dst nodes are
    RELABELED via degree-balanced LPT binning into 8 cores x 49 windows of
    128 nodes, so every (core, window) has a near-equal edge count and chunk
    padding is <1%.  Per core, edges sorted by dst window and packed into
    128-edge chunks (one matmul each).  dma_gather uses int16 indices, so
    the node table is addressed via two 32768-row views (LOW/HIGH); srcs in
    the overlap may go to either section and are assigned to fill chunk
    boundaries exactly.  Each window's LOW and HIGH chunks run back to
    back so its PSUM accumulation closes in one pass (single eviction,
    epilogues spread across the phase).  Biases are folded away when zero
    (relu(dinv*x) = dinv*relu(x); dinv^2 in the final replicate-scale).
    Self-loop edges are NOT gathered: the permuted
    table makes each window's own rows contiguous, so they stream in with a
    plain DMA and join the accumulation as one identity-operand matmul.
  - Device per core:
      Gathers are issued round-robin on the 4 SWDGE queues (num_swdge_
      queues=4).  Descriptor generation for queues 1-3 runs on their own Q7
      core pairs concurrently under queue 0's engine-blocking gather,
      cutting effective desc-gen from 8.4 ns/idx to ~2.6 ns/idx.
      Phase A (layer 1): dma_gather source rows of the prescaled x-table ->
        G [128e, d_in]; build one-hot S [128e, 128dst] on DVE (bf16 iota ==
        dst_rel); PE matmul accumulates G.T @ S into PSUM [d_in, 128dst]
        per window.  Per window: x W1 (PE), scale by dinv[dst], +b1, ReLU;
        transpose (PE); x W2; scale by dinv[dst]; replicate 64x -> 256B
        rows of the h2 table, DMA out.
      AllGather h2 shards -> full replicated table.
      Phase B (layer 2): same chunk structure; gather h2 rows, matmul
        S.T @ G2[:, :2] accumulated per window; scale by dinv[dst], +b2.
    Output rows are in permuted order; the host scatters them back.
"""

import numpy as np

import concourse.bass as bass
import concourse.mybir as mybir
import concourse.tile as tile
from concourse import bacc
from concourse.bass_utils import run_bass_kernel_spmd

F32 = mybir.dt.float32
BF16 = mybir.dt.bfloat16
I16 = mybir.dt.int16

N_CORES = 8
WINDOW = 128  # dst nodes per PSUM accumulation window
CHUNK = 128  # edges per matmul chunk
GSZ = 8  # max chunks per dma_gather instruction (1024 idxs, single-packet)
SBATCH = 8  # chunks per S-build DVE op
HALF = 32768  # int16 index range
REP = 64  # h2 replication (64x2 bf16 cols -> 256B rows)
W_SPLITS = (0, 25, 49)  # h2 AllGather pieces: window ranges, exchanged as each closes
GATHER_BF16 = True  # layer-1 gather table + chunk matmuls in bf16


# --------------------------------------------------------------------------
# Host preprocessing
# --------------------------------------------------------------------------
def _preprocess(x, edge_index, n_cores):
    N = x.shape[0]
    src = np.asarray(edge_index[0], dtype=np.int64)
    dst = np.asarray(edge_index[1], dtype=np.int64)
    # degree includes the self-loop (handled separately on-device)
    deg = (np.bincount(dst, minlength=N) + 1).astype(np.float64)
    dinv = np.where(deg > 0, 1.0 / np.sqrt(deg), 0.0).astype(np.float32)

    # ---- degree-balanced dst binning: 392 bins (8 cores x 49 windows), ----
    # ---- capacity 128 nodes each; LPT greedy on in-degree ----
    import heapq

    w_cnt = 49
    n_bins = n_cores * w_cnt
    n_local = w_cnt * WINDOW  # 6272 (padded shard; empty slots get dinv 0)
    deg_ns = np.bincount(dst, minlength=N).astype(np.float64)
    order_nodes = np.argsort(-deg_ns, kind="stable")
    heap = [(0.0, b, 0) for b in range(n_bins)]  # (edge_sum, bin, n_nodes)
    heapq.heapify(heap)
    bin_of = np.empty(N, dtype=np.int64)
    slot_of = np.empty(N, dtype=np.int64)
    stash = []
    for n in order_nodes:
        while True:
            s_, b, cnt = heapq.heappop(heap)
            if cnt < WINDOW:
                break
            stash.append((s_, b, cnt))  # full bin, drop
        bin_of[n] = b
        slot_of[n] = cnt
        heapq.heappush(heap, (s_ + deg_ns[n], b, cnt + 1))
    # permuted position of node n
    permpos = bin_of * WINDOW + slot_of  # in [0, 50176)
    # inverse: node at permuted position p (or -1)
    inv = np.full(n_cores * n_local, -1, dtype=np.int64)
    inv[permpos] = np.arange(N)

    # table rows: 0 = zero, 1..NP = permuted slots, NP+1 = zero.
    # row(node n) = permpos[n] + 1
    # LOW view = rows [0, min(HALF, NP+2)); HIGH view = rows [HB, HB+HALF)
    NP = n_cores * n_local
    HB = max(0, NP + 2 - HALF)
    # section 0 restricted to the first AllGather region so phase B's
    # first pass can gather while the second exchange piece is in flight
    lowmax_row = min(HALF, NP + 2, 1 + n_cores * W_SPLITS[1] * WINDOW)
    pad_low = 0
    pad_high = NP + 1 - HB

    # table layout: row 0 = zero; A-region rows [1, 1+nc*ASZ) hold the
    # first A_WIN windows of every shard (early AllGather); B-region rows
    # hold the rest; last row = zero.  rowof(p) maps permuted position.
    p_c = permpos // n_local
    p_off = permpos % n_local
    tabrow_of_pos = np.zeros(N, dtype=np.int64)
    roff = 1
    for r in range(len(W_SPLITS) - 1):
        lo, hi = W_SPLITS[r] * WINDOW, W_SPLITS[r + 1] * WINDOW
        rsz = hi - lo
        m = (p_off >= lo) & (p_off < hi)
        tabrow_of_pos[m] = roff + p_c[m] * rsz + (p_off[m] - lo)
        roff += n_cores * rsz
    rows_all = tabrow_of_pos[src]  # gather row per edge
    dstbin = bin_of[dst]
    dstslot = slot_of[dst]

    # group edges by dst bin
    eorder = np.argsort(dstbin, kind="stable")
    g_rows = rows_all[eorder]
    g_slot = dstslot[eorder]
    g_bin = dstbin[eorder]
    starts = np.searchsorted(g_bin, np.arange(n_bins + 1))

    # per (core, window): split edges into LOW/HIGH with flex edges
    # (rows in [HB, lowmax) fit either view). kw must be uniform across
    # cores, so pick per-window section targets jointly: kw_lo = max_c
    # ceil(lo_only/128), all cores fill LOW to that boundary with flex.
    split = {}
    for b in range(n_bins):
        c, w = divmod(b, w_cnt)
        rows = g_rows[starts[b] : starts[b + 1]]
        rel = g_slot[starts[b] : starts[b + 1]].astype(np.float32)
        is_lo_only = rows < HB
        is_hi_only = rows >= lowmax_row
        is_flex = ~is_lo_only & ~is_hi_only
        split[(c, w)] = (
            (rows[is_lo_only], rel[is_lo_only]),
            (rows[is_hi_only], rel[is_hi_only]),
            (rows[is_flex], rel[is_flex]),
        )
    parts = {}
    kw_lo = np.zeros(w_cnt, dtype=np.int64)
    kw_hi = np.zeros(w_cnt, dtype=np.int64)
    for w in range(w_cnt):
        klo_a = max(
            1, max(-(-len(split[(c, w)][0][0]) // CHUNK) for c in range(n_cores))
        )
        khi_a = max(
            -(
                -(len(split[(c, w)][1][0]) + max(
                    0,
                    len(split[(c, w)][2][0])
                    - (klo_a * CHUNK - len(split[(c, w)][0][0])),
                ))
                // CHUNK
            )
            for c in range(n_cores)
        )
        khi_a = max(1, khi_a)
        # alternative: minimize HIGH first
        khi_b = max(
            1, max(-(-len(split[(c, w)][1][0]) // CHUNK) for c in range(n_cores))
        )
        klo_b = max(
            -(
                -(len(split[(c, w)][0][0]) + max(
                    0,
                    len(split[(c, w)][2][0])
                    - (khi_b * CHUNK - len(split[(c, w)][1][0])),
                ))
                // CHUNK
            )
            for c in range(n_cores)
        )
        klo_b = max(1, klo_b)
        if klo_a + khi_a <= klo_b + khi_b:
            kw_lo[w], kw_hi[w], fill_low = klo_a, khi_a, True
        else:
            kw_lo[w], kw_hi[w], fill_low = klo_b, khi_b, False
        for c in range(n_cores):
            (lo_r, lo_s), (hi_r, hi_s), (fx_r, fx_s) = split[(c, w)]
            if fill_low:
                take = min(len(fx_r), kw_lo[w] * CHUNK - len(lo_r))
            else:
                take = len(fx_r) - min(len(fx_r), kw_hi[w] * CHUNK - len(hi_r))
            parts[(c, w, 0)] = (
                np.concatenate([lo_r, fx_r[:take]]),
                np.concatenate([lo_s, fx_s[:take]]),
            )
            parts[(c, w, 1)] = (
                np.concatenate([hi_r, fx_r[take:]]) - HB,
                np.concatenate([hi_s, fx_s[take:]]),
            )
    T_lo, T_hi = int(kw_lo.sum()), int(kw_hi.sum())
    T = T_lo + T_hi

    # chunk order: per window, LOW chunks then HIGH chunks (PSUM stays
    # open across both sections; one eviction per window).
    chunk_win = []  # (window, first_overall, last_overall, section)
    for w in range(w_cnt):
        kt = int(kw_lo[w] + kw_hi[w])
        k = 0
        for sec, kws in ((0, kw_lo), (1, kw_hi)):
            for _ in range(int(kws[w])):
                chunk_win.append((w, k == 0, k == kt - 1, sec))
                k += 1

    per_core = []
    for c in range(n_cores):
        idx_lin = np.zeros(T * CHUNK, dtype=np.int32)
        dstrel = np.zeros((CHUNK, T), dtype=np.float32)
        t = 0
        for w in range(w_cnt):
            for sec, kws, padrow in ((0, kw_lo, pad_low), (1, kw_hi, pad_high)):
                rows, rel = parts[(c, w, sec)]
                n_e = len(rows)
                n_slots = int(kws[w]) * CHUNK
                buf = np.full(n_slots, padrow, dtype=np.int32)
                buf[:n_e] = rows
                idx_lin[t * CHUNK : t * CHUNK + n_slots] = buf
                rbuf = np.zeros(n_slots, dtype=np.float32)
                rbuf[:n_e] = rel
                dstrel[:, t : t + int(kws[w])] = rbuf.reshape(int(kws[w]), CHUNK).T
                t += int(kws[w])
        assert t == T
        idx16 = idx_lin.astype(np.int16).reshape(T * CHUNK // 16, 16).T  # [16, S]
        idx16 = np.tile(idx16, (8, 1))  # [128, S]

        dinvw = np.zeros((WINDOW, w_cnt), dtype=np.float32)
        for w in range(w_cnt):
            nodes = inv[(c * w_cnt + w) * WINDOW : (c * w_cnt + w + 1) * WINDOW]
            valid = nodes >= 0
            dinvw[valid, w] = dinv[nodes[valid]]
        per_core.append({"idx16": idx16, "dstrel": dstrel, "dinvw": dinvw})

    kw_tot = kw_lo + kw_hi
    t_agsplit = [int(kw_tot[: W_SPLITS[r + 1]].sum()) for r in range(len(W_SPLITS) - 2)]

    return {
        "t_agsplit": t_agsplit,
        "tabrow_of_pos": tabrow_of_pos,
        "n_local": n_local,
        "w_cnt": w_cnt,
        "kw_lo": kw_lo,
        "kw_hi": kw_hi,
        "T_lo": T_lo,
        "T_hi": T_hi,
        "T": T,
        "HB": HB,
        "chunk_win": chunk_win,
        "dinv": dinv,
        "permpos": permpos,
        "inv": inv,
        "per_core": per_core,
    }


# --------------------------------------------------------------------------
# Device kernel builder (one program, SPMD across cores)
# --------------------------------------------------------------------------
def _assign_queues(sizes, w0=0.20):
    """Greedy queue assignment: queue 0's desc-gen blocks the gpsimd engine,
    queues 1-3 run async on their own Q7 pairs - so give q0 a smaller share."""
    weights = [w0, (1.0 - w0) / 3, (1.0 - w0) / 3, (1.0 - w0) / 3]
    load = [0.0, 0.0, 0.0, 0.0]
    out = []
    for sz in sizes:
        q = min(range(4), key=lambda k: (load[k] + sz) / weights[k])
        load[q] += sz
        out.append(q)
    return out


def _build(nc, *, N, n_local, d_in, d_hid, n_cls, pp, n_cores, dt_gat, bias_zero, bias2_zero):
    Relu = mybir.ActivationFunctionType.Relu
    Copy = mybir.ActivationFunctionType.Copy
    T, T_lo = pp["T"], pp["T_lo"]
    w_cnt, HB = pp["w_cnt"], pp["HB"]
    chunk_win = pp["chunk_win"]
    d_rep = REP * n_cls  # 64 cols of f32 -> 256B rows

    xtab = nc.dram_tensor("xtab", [N + 2, d_in], dt_gat, kind="ExternalInput")
    w1 = nc.dram_tensor("w1", [d_in, d_hid], F32, kind="ExternalInput")
    w2 = nc.dram_tensor("w2", [d_hid, n_cls], F32, kind="ExternalInput")
    b1bc = nc.dram_tensor("b1bc", [WINDOW, d_hid], F32, kind="ExternalInput")
    b2bc = nc.dram_tensor("b2bc", [WINDOW, n_cls], F32, kind="ExternalInput")
    iota = nc.dram_tensor("iota", [CHUNK, SBATCH * WINDOW], BF16, kind="ExternalInput")
    ident = nc.dram_tensor("ident", [WINDOW, WINDOW], F32, kind="ExternalInput")
    identb = nc.dram_tensor("identb", [WINDOW, WINDOW], dt_gat, kind="ExternalInput")
    xself = nc.dram_tensor("xself", [n_local, d_in], dt_gat, kind="ExternalInput")
    idx_t = nc.dram_tensor("idx16", [CHUNK, T * 8], I16, kind="ExternalInput")
    dstrel_t = nc.dram_tensor("dstrel", [CHUNK, T], BF16, kind="ExternalInput")
    dinvw_t = nc.dram_tensor("dinvw", [WINDOW, w_cnt], F32, kind="ExternalInput")
    out_t = nc.dram_tensor("out", [n_local, n_cls], F32, kind="ExternalOutput")

    h2loc = nc.dram_tensor("h2loc", [n_local, d_rep], BF16)
    h2tab = nc.dram_tensor("h2tab", [N + 2, d_rep], BF16, addr_space="Shared")

    # gather groups: runs of chunks sharing (window, section), split into
    # near-equal pieces of <= GSZ; round-robin across the 4 SWDGE queues.
    groups = []
    t0 = 0
    while t0 < T:
        w0, _, _, sec0 = chunk_win[t0]
        t1 = t0
        while t1 < T and chunk_win[t1][0] == w0 and chunk_win[t1][3] == sec0:
            t1 += 1
        run = t1 - t0
        n_pieces = -(-run // GSZ)
        base = run // n_pieces
        rem = run % n_pieces
        tt0 = t0
        for i in range(n_pieces):
            n = base + (1 if i < rem else 0)
            groups.append((sec0, tt0, n))
            tt0 += n
        t0 = t1
    qa = _assign_queues([g[2] for g in groups])
    groups = [(sec0, tt0, n, q) for (sec0, tt0, n), q in zip(groups, qa)]

    # (row offset into h2tab/xtab, local row range) per AllGather piece
    regions = []
    roff = 1
    for r in range(len(W_SPLITS) - 1):
        lo, hi = W_SPLITS[r] * WINDOW, W_SPLITS[r + 1] * WINDOW
        regions.append((roff, lo, hi))
        roff += n_cores * (hi - lo)
    t_agsplit = pp["t_agsplit"]

    def emit_exchange(r):
        roff_r, lo, hi = regions[r]
        if n_cores > 1:
            nc.gpsimd.collective_compute(
                "AllGather",
                mybir.AluOpType.bypass,
                replica_groups=[list(range(n_cores))],
                ins=[h2loc[lo:hi, :]],
                outs=[h2tab[roff_r : roff_r + n_cores * (hi - lo), :]],
            )
        else:
            nc.sync.dma_start(
                out=h2tab[roff_r : roff_r + (hi - lo), :], in_=h2loc[lo:hi, :]
            )

    def tab_view(tab):
        return [
            tab[0 : min(HALF, N + 2), :],
            tab[HB : min(HB + HALF, N + 2), :],
        ]

    with tile.TileContext(nc) as tc:
        with (
            tc.tile_pool(name="const", bufs=1) as cpool,
            tc.tile_pool(name="gbuf", bufs=12) as gpool,
            tc.tile_pool(name="g2buf", bufs=12) as g2pool,
            tc.tile_pool(name="sbat", bufs=8) as spool,
            tc.tile_pool(name="sbat2", bufs=8) as s2pool,
            tc.tile_pool(name="wtmp", bufs=3) as wpool,
            tc.tile_pool(name="selfp", bufs=3) as selfpool,
            tc.tile_pool(name="psA", bufs=4, space="PSUM") as psA,
            tc.tile_pool(name="psW", bufs=4, space="PSUM") as psW,
        ):
            # ---- constants into SBUF ----
            w1_sb = cpool.tile([d_in, d_hid], F32, tag="w1")
            nc.sync.dma_start(out=w1_sb[:], in_=w1[:])
            w2_sb = cpool.tile([d_hid, n_cls], F32, tag="w2")
            nc.sync.dma_start(out=w2_sb[:], in_=w2[:])
            b1_sb = cpool.tile([WINDOW, d_hid], F32, tag="b1")
            nc.sync.dma_start(out=b1_sb[:], in_=b1bc[:])
            b2_sb = cpool.tile([WINDOW, n_cls], F32, tag="b2")
            nc.sync.dma_start(out=b2_sb[:], in_=b2bc[:])
            iota_sb = cpool.tile([CHUNK, SBATCH * WINDOW], BF16, tag="iota")
            nc.sync.dma_start(out=iota_sb[:], in_=iota[:])
            id_sb = cpool.tile([WINDOW, WINDOW], F32, tag="ident")
            nc.sync.dma_start(out=id_sb[:], in_=ident[:])
            idb_sb = cpool.tile([WINDOW, WINDOW], dt_gat, tag="identb")
            nc.sync.dma_start(out=idb_sb[:], in_=identb[:])
            idx_sb = cpool.tile([CHUNK, T * 8], I16, tag="idx")
            nc.sync.dma_start(out=idx_sb[:], in_=idx_t[:])
            dstrel_sb = cpool.tile([CHUNK, T], BF16, tag="dstrel")
            nc.sync.dma_start(out=dstrel_sb[:], in_=dstrel_t[:])
            dinvw_sb = cpool.tile([WINDOW, w_cnt], F32, tag="dinvw")
            nc.sync.dma_start(out=dinvw_sb[:], in_=dinvw_t[:])
            d2_sb = cpool.tile([WINDOW, w_cnt], F32, tag="dinv2")
            nc.vector.tensor_tensor(
                out=d2_sb[:], in0=dinvw_sb[:], in1=dinvw_sb[:],
                op=mybir.AluOpType.mult,
            )

            zrow = cpool.tile([1, d_rep], BF16, tag="zrow")
            nc.vector.memset(zrow[:], 0.0)
            nc.sync.dma_start(out=h2tab[0:1, :], in_=zrow[:1, :])
            nc.sync.dma_start(out=h2tab[N + 1 : N + 2, :], in_=zrow[:1, :])

            def build_s(pool, t0, n, nm):
                """one-hot S for chunks [t0, t0+n) in one DVE op."""
                s_tile = pool.tile([CHUNK, SBATCH * WINDOW], BF16, tag="s", name=nm)
                rel_b = (
                    dstrel_sb[:, t0 : t0 + n]
                    .rearrange("p (b one) -> p b one", one=1)
                    .to_broadcast([CHUNK, n, WINDOW])
                )
                io_v = iota_sb[:, : n * WINDOW].rearrange("p (b j) -> p b j", j=WINDOW)
                s_v = s_tile[:, : n * WINDOW].rearrange("p (b j) -> p b j", j=WINDOW)
                nc.vector.tensor_tensor(
                    out=s_v, in0=io_v, in1=rel_b, op=mybir.AluOpType.is_equal
                )
                return s_tile

            # =========================== PHASE A ===========================
            psum_of_win = {}
            for sec, t0, n, qn in groups:
                gb = gpool.tile([CHUNK, GSZ, d_in], dt_gat, tag="g", name="gb")
                nc.gpsimd.dma_gather(
                    gb[:, :n, :],
                    tab_view(xtab)[sec],
                    idx_sb[:, t0 * 8 : (t0 + n) * 8],
                    n * CHUNK,
                    n * CHUNK,
                    d_in,
                    single_packet=True,
                    queue_num=qn,
                )
                for bt0 in range(t0, t0 + n, SBATCH):
                    bn = min(SBATCH, t0 + n - bt0)
                    s_tile = build_s(spool, bt0, bn, "sA")
                    for t in range(bt0, bt0 + bn):
                        j = t - bt0
                        w, first, last, _sec = chunk_win[t]
                        if first:
                            psum_of_win[w] = psA.tile(
                                [d_in, WINDOW], F32, tag="agg", name="aggps"
                            )
                            xw = selfpool.tile([WINDOW, d_in], dt_gat, tag="xw")
                            nc.sync.dma_start(
                                out=xw[:],
                                in_=xself[w * WINDOW : (w + 1) * WINDOW, :],
                            )
                            nc.tensor.matmul(
                                out=psum_of_win[w][:],
                                lhsT=xw[:],
                                rhs=idb_sb[:],
                                start=True,
                                stop=False,
                            )
                        nc.tensor.matmul(
                            out=psum_of_win[w][:],
                            lhsT=gb[:, t - t0, :],
                            rhs=s_tile[:, j * WINDOW : (j + 1) * WINDOW],
                            start=False,
                            stop=last,
                        )
                        if not last:
                            continue
                        ps = psum_of_win.pop(w)
                        aggw = wpool.tile([d_in, WINDOW], F32, tag="aggw")
                        nc.scalar.activation(out=aggw[:], in_=ps[:], func=Copy)
                        _window_epilogue_A(
                            nc, w, aggw[:], wpool, psW, w1_sb, w2_sb, b1_sb,
                            dinvw_sb, d2_sb, id_sb, idb_sb, h2loc, n_local,
                            d_in, d_hid, n_cls, d_rep, bias_zero,
                        )
                if t0 + n in t_agsplit:
                    # a window range just closed: exchange its h2 rows now,
                    # overlapping the collective with the remaining gathers
                    emit_exchange(t_agsplit.index(t0 + n))

            # ==== h2 exchange, last piece (earlier pieces were emitted ====
            # ==== mid-phase-A and hide under the remaining gathers) ====
            emit_exchange(len(regions) - 1)

            # =========================== PHASE B ===========================
            # Section-major: pass 0 gathers only rows < AVIEW (the region
            # exchanged by the FIRST AllGather piece), so its gathers flow
            # straight out of phase A while the second piece is in flight;
            # pass 1 (HIGH view) accumulates on top of the pass-0 partials.
            kw_lo_a = pp["kw_lo"]
            kw_hi_a = pp["kw_hi"]
            chunk_base = [0] * (w_cnt + 1)
            for w in range(w_cnt):
                chunk_base[w + 1] = chunk_base[w] + int(kw_lo_a[w] + kw_hi_a[w])
            AVIEW = 1 + n_cores * W_SPLITS[1] * WINDOW
            out2_sb = cpool.tile([WINDOW, w_cnt * n_cls], F32, tag="out2")
            bgroups = []
            for sec in (0, 1):
                for w in range(w_cnt):
                    t0w = chunk_base[w] + (0 if sec == 0 else int(kw_lo_a[w]))
                    run = int(kw_lo_a[w]) if sec == 0 else int(kw_hi_a[w])
                    n_pieces = -(-run // GSZ)
                    gbase = run // n_pieces
                    grem = run % n_pieces
                    tt0 = t0w
                    for i in range(n_pieces):
                        n = gbase + (1 if i < grem else 0)
                        bgroups.append((sec, w, tt0, n))
                        tt0 += n
            qb = _assign_queues([g[3] for g in bgroups])
            bgroups = [(sec, w, tt0, n, q) for (sec, w, tt0, n), q in zip(bgroups, qb)]
            psum_of_win = {}
            for sec, w, t0, n, qn in bgroups:
                g2 = g2pool.tile([CHUNK, GSZ, d_rep], BF16, tag="g2", name="g2b")
                view = h2tab[0:AVIEW, :] if sec == 0 else tab_view(h2tab)[1]
                nc.gpsimd.dma_gather(
                    g2[:, :n, :],
                    view,
                    idx_sb[:, t0 * 8 : (t0 + n) * 8],
                    n * CHUNK,
                    n * CHUNK,
                    d_rep,
                    single_packet=True,
                    queue_num=qn,
                )
                for bt0 in range(t0, t0 + n, SBATCH):
                    bn = min(SBATCH, t0 + n - bt0)
                    s_tile = build_s(s2pool, bt0, bn, "sB")
                    for t in range(bt0, bt0 + bn):
                        j = t - bt0
                        sec0_first = sec == 0 and t == chunk_base[w]
                        sec0_last = (
                            sec == 0 and t == chunk_base[w] + int(kw_lo_a[w]) - 1
                        )
                        sec1_first = (
                            sec == 1 and t == chunk_base[w] + int(kw_lo_a[w])
                        )
                        sec1_last = sec == 1 and t == chunk_base[w + 1] - 1
                        if sec0_first:
                            psum_of_win[w] = psA.tile(
                                [WINDOW, n_cls], F32, tag="agg", name="agg2ps"
                            )
                            h2w = selfpool.tile([WINDOW, d_rep], BF16, tag="h2w")
                            nc.sync.dma_start(
                                out=h2w[:],
                                in_=h2loc[w * WINDOW : (w + 1) * WINDOW, :],
                            )
                            nc.tensor.matmul(
                                out=psum_of_win[w][:],
                                lhsT=idb_sb[:],
                                rhs=h2w[:, :n_cls],
                                start=True,
                                stop=False,
                            )
                        elif sec1_first:
                            psum_of_win[w] = psA.tile(
                                [WINDOW, n_cls], F32, tag="agg", name="agg2ps"
                            )
                        nc.tensor.matmul(
                            out=psum_of_win[w][:],
                            lhsT=s_tile[:, j * WINDOW : (j + 1) * WINDOW],
                            rhs=g2[:, t - t0, :n_cls],
                            start=sec1_first,
                            stop=sec0_last or sec1_last,
                        )
                        osl = out2_sb[:, w * n_cls : (w + 1) * n_cls]
                        if sec0_last:
                            ps = psum_of_win.pop(w)
                            nc.scalar.activation(out=osl, in_=ps[:], func=Copy)
                        elif sec1_last:
                            ps = psum_of_win.pop(w)
                            ob = wpool.tile([WINDOW, n_cls], F32, tag="ob")
                            nc.vector.tensor_tensor(
                                out=ob[:], in0=ps[:], in1=osl,
                                op=mybir.AluOpType.add,
                            )
                            ob2 = wpool.tile([WINDOW, n_cls], F32, tag="ob2")
                            nc.vector.tensor_scalar(
                                out=ob2[:],
                                in0=ob[:],
                                scalar1=dinvw_sb[:, w : w + 1],
                                scalar2=None,
                                op0=mybir.AluOpType.mult,
                            )
                            ob3 = ob2
                            if not bias2_zero:
                                ob3 = wpool.tile([WINDOW, n_cls], F32, tag="ob3")
                                nc.vector.tensor_tensor(
                                    out=ob3[:], in0=ob2[:], in1=b2_sb[:],
                                    op=mybir.AluOpType.add,
                                )
                            nrows = min(WINDOW, n_local - w * WINDOW)
                            nc.sync.dma_start(
                                out=out_t[w * WINDOW : w * WINDOW + nrows, :],
                                in_=ob3[:nrows, :],
                            )

    nc.compile()
    return nc


def _window_epilogue_A(
    nc, w, aggT, wpool, psW, w1_sb, w2_sb, b1_sb, dinvw_sb, d2_sb, id_sb,
    idb_sb, h2loc, n_local, d_in, d_hid, n_cls, d_rep, bias_zero,
):
    """aggT [d_in, WINDOW] in SBUF -> replicated h2 rows in DRAM."""
    Relu = mybir.ActivationFunctionType.Relu
    Copy = mybir.ActivationFunctionType.Copy

    # h1 [dst, hid] = aggT.T @ W1
    h1_ps = psW.tile([WINDOW, d_hid], F32, tag="wps", name="h1_ps")
    nc.tensor.matmul(out=h1_ps[:], lhsT=aggT, rhs=w1_sb[:], start=True, stop=True)
    if bias_zero:
        # b1 == 0: relu(dinv*x) = dinv*relu(x); relu straight off PSUM and
        # fold both dinv factors into the final replicate-scale (dinv^2).
        r3_sb = wpool.tile([WINDOW, d_hid], F32, tag="r3")
        nc.scalar.activation(out=r3_sb[:], in_=h1_ps[:], func=Relu)
        fin_scale = d2_sb
    else:
        r_sb = wpool.tile([WINDOW, d_hid], F32, tag="r")
        nc.vector.tensor_scalar(
            out=r_sb[:],
            in0=h1_ps[:],
            scalar1=dinvw_sb[:, w : w + 1],
            scalar2=None,
            op0=mybir.AluOpType.mult,
        )
        r2_sb = wpool.tile([WINDOW, d_hid], F32, tag="r2")
        nc.vector.tensor_tensor(
            out=r2_sb[:], in0=r_sb[:], in1=b1_sb[:], op=mybir.AluOpType.add
        )
        r3_sb = wpool.tile([WINDOW, d_hid], F32, tag="r3")
        nc.scalar.activation(out=r3_sb[:], in_=r2_sb[:], func=Relu)
        fin_scale = dinvw_sb
    # transpose -> [hid, dst]
    rT_ps = psW.tile([d_hid, WINDOW], F32, tag="wps", name="rT_ps")
    nc.tensor.transpose(out=rT_ps[:], in_=r3_sb[:], identity=id_sb[:])
    rT_sb = wpool.tile([d_hid, WINDOW], F32, tag="rTs")
    nc.scalar.activation(out=rT_sb[:], in_=rT_ps[:], func=Copy)
    # h2 [dst, n_cls] = rT.T @ W2; scale; replicate REP x
    h2_ps = psW.tile([WINDOW, n_cls], F32, tag="wps", name="h2_ps")
    nc.tensor.matmul(out=h2_ps[:], lhsT=rT_sb[:], rhs=w2_sb[:], start=True, stop=True)
    h2_sb = wpool.tile([WINDOW, d_rep], BF16, tag="h2s")
    nc.vector.tensor_scalar(
        out=h2_sb[:].rearrange("p (r c) -> p r c", c=n_cls),
        in0=h2_ps[:]
        .rearrange("p (one c) -> p one c", one=1)
        .to_broadcast([WINDOW, REP, n_cls]),
        scalar1=fin_scale[:, w : w + 1],
        scalar2=None,
        op0=mybir.AluOpType.mult,
    )
    nrows = min(WINDOW, n_local - w * WINDOW)
    nc.sync.dma_start(
        out=h2loc[w * WINDOW : w * WINDOW + nrows, :], in_=h2_sb[:nrows, :]
    )


# --------------------------------------------------------------------------
# Entry point
# --------------------------------------------------------------------------
def _make_inputs(x, W1, b1, W2, b2, pp, dt_np, n_cores):
    N, d_in = x.shape
    NP = n_cores * pp["n_local"]
    W1 = np.asarray(W1, np.float32)
    b1 = np.asarray(b1, np.float32)
    W2 = np.asarray(W2, np.float32)
    b2 = np.asarray(b2, np.float32)
    d_hid = W1.shape[1]
    n_cls = W2.shape[1]
    xp = x * pp["dinv"][:, None]
    xtab = np.zeros((NP + 2, d_in), np.float32)
    xtab[pp["tabrow_of_pos"]] = xp
    xtab = xtab.astype(dt_np)
    iota_arr = np.broadcast_to(
        np.tile(np.arange(WINDOW, dtype=np.float32), SBATCH),
        (CHUNK, SBATCH * WINDOW),
    ).astype(np.dtype("bfloat16"))
    shared = {
        "xtab": xtab,
        "w1": W1,
        "w2": W2,
        "b1bc": np.broadcast_to(b1, (WINDOW, d_hid)).astype(np.float32).copy(),
        "b2bc": np.broadcast_to(b2, (WINDOW, n_cls)).astype(np.float32).copy(),
        "iota": iota_arr,
        "ident": np.eye(WINDOW, dtype=np.float32),
        "identb": np.eye(WINDOW, dtype=np.float32).astype(dt_np),
    }
    n_local = pp["n_local"]
    in_maps = []
    for c, pc in enumerate(pp["per_core"]):
        m = dict(shared)
        xs = np.zeros((n_local, d_in), np.float32)
        sl = pp["inv"][c * n_local : (c + 1) * n_local]
        xs[sl >= 0] = xp[sl[sl >= 0]]
        m["xself"] = xs.astype(dt_np)
        m["idx16"] = pc["idx16"]
        m["dstrel"] = pc["dstrel"].astype(np.dtype("bfloat16"))
        m["dinvw"] = pc["dinvw"]
        in_maps.append(m)
    return in_maps


def _run(x, edge_index, W1, b1, W2, b2, n_cores, trace=False):
    x = np.asarray(x, dtype=np.float32)
    N, d_in = x.shape
    d_hid = np.asarray(W1).shape[1]
    n_cls = np.asarray(W2).shape[1]
    assert d_in == 128 and d_hid == 128

    pp = _preprocess(x, edge_index, n_cores)
    dt_gat = BF16 if GATHER_BF16 else F32
    np_gat = np.dtype("bfloat16") if GATHER_BF16 else np.dtype("float32")

    nc = bacc.Bacc("TRN2", target_bir_lowering=False, debug=False, num_swdge_queues=4)
    _build(
        nc,
        N=n_cores * pp["n_local"],
        n_local=pp["n_local"],
        d_in=d_in,
        d_hid=d_hid,
        n_cls=n_cls,
        pp=pp,
        n_cores=n_cores,
        dt_gat=dt_gat,
        bias_zero=bool(np.all(np.asarray(b1) == 0)),
        bias2_zero=bool(np.all(np.asarray(b2) == 0)),
    )

    import ml_dtypes  # noqa

    in_maps = _make_inputs(x, W1, b1, W2, b2, pp, np_gat, n_cores)
    res = run_bass_kernel_spmd(nc, in_maps, list(range(n_cores)), trace=trace)
    outs = [res.results[c]["out"] for c in range(n_cores)]
    full = np.concatenate(outs, axis=0)[pp["permpos"]]
    return full.astype(np.float32), res


def kernel(x, edge_index, W1, b1, W2, b2):
    out, _ = _run(x, edge_index, W1, b1, W2, b2, N_CORES)
    return out



# revision 33
# speedup vs baseline: 1.0764x; 1.0634x over previous
"""GCN 2-layer (PyG GCNConv x2 + ReLU) Bass kernel for Trainium2, 8-core SPMD.

Strategy:
  - Host: compute symmetric normalization dinv = deg^-1/2 (self-loop incl.),
    fold dinv[src] into a prescaled gather table (x * dinv).  Dst nodes are
    RELABELED via degree-balanced LPT binning into 8 cores x 49 windows of
    128 nodes, so every (core, window) has a near-equal edge count and chunk
    padding is <1%.  Per core, edges sorted by dst window and packed into
    128-edge chunks (one matmul each).  dma_gather uses int16 indices, so
    the node table is addressed via two 32768-row views (LOW/HIGH); srcs in
    the overlap may go to either section and are assigned to fill chunk
    boundaries exactly.  Each window's LOW and HIGH chunks run back to
    back so its PSUM accumulation closes in one pass (single eviction,
    epilogues spread across the phase).  Biases are folded away when zero
    (relu(dinv*x) = dinv*relu(x); dinv^2 in the final replicate-scale).
    Self-loop edges are NOT gathered: the permuted
    table makes each window's own rows contiguous, so they stream in with a
    plain DMA and join the accumulation as one identity-operand matmul.
  - Device per core:
      Gathers are issued round-robin on the 4 SWDGE queues (num_swdge_
      queues=4).  Descriptor generation for queues 1-3 runs on their own Q7
      core pairs concurrently under queue 0's engine-blocking gather,
      cutting effective desc-gen from 8.4 ns/idx to ~2.6 ns/idx.
      Phase A (layer 1): dma_gather source rows of the prescaled x-table ->
        G [128e, d_in]; build one-hot S [128e, 128dst] on DVE (bf16 iota ==
        dst_rel); PE matmul accumulates G.T @ S into PSUM [d_in, 128dst]
        per window.  Per window: x W1 (PE), scale by dinv[dst], +b1, ReLU;
        transpose (PE); x W2; scale by dinv[dst]; replicate 64x -> 256B
        rows of the h2 table, DMA out.
      AllGather h2 shards -> full replicated table.
      Phase B (layer 2): same chunk structure; gather h2 rows, matmul
        S.T @ G2[:, :2] accumulated per window; scale by dinv[dst], +b2.
    Output rows are in permuted order; the host scatters them back.
"""

import numpy as np

import concourse.bass as bass
import concourse.mybir as mybir
import concourse.tile as tile
from concourse import bacc
from concourse.bass_utils import run_bass_kernel_spmd

F32 = mybir.dt.float32
BF16 = mybir.dt.bfloat16
I16 = mybir.dt.int16

N_CORES = 8
WINDOW = 128  # dst nodes per PSUM accumulation window
CHUNK = 128  # edges per matmul chunk
GSZ = 8  # max chunks per dma_gather instruction (1024 idxs, single-packet)
SBATCH = 8  # chunks per S-build DVE op
HALF = 32768  # int16 index range
REP = 64  # h2 replication (64x2 bf16 cols -> 256B rows)
W_SPLITS = (0, 25, 49)  # h2 AllGather pieces: window ranges, exchanged as each closes
GATHER_BF16 = True  # layer-1 gather table + chunk matmuls in bf16


# --------------------------------------------------------------------------
# Host preprocessing
# --------------------------------------------------------------------------
def _preprocess(x, edge_index, n_cores):
    N = x.shape[0]
    src = np.asarray(edge_index[0], dtype=np.int64)
    dst = np.asarray(edge_index[1], dtype=np.int64)
    # degree includes the self-loop (handled separately on-device)
    deg = (np.bincount(dst, minlength=N) + 1).astype(np.float64)
    dinv = np.where(deg > 0, 1.0 / np.sqrt(deg), 0.0).astype(np.float32)

    # ---- degree-balanced dst binning: 392 bins (8 cores x 49 windows), ----
    # ---- capacity 128 nodes each; LPT greedy on in-degree ----
    import heapq

    w_cnt = 49
    n_bins = n_cores * w_cnt
    n_local = w_cnt * WINDOW  # 6272 (padded shard; empty slots get dinv 0)
    deg_ns = np.bincount(dst, minlength=N).astype(np.float64)
    order_nodes = np.argsort(-deg_ns, kind="stable")
    heap = [(0.0, b, 0) for b in range(n_bins)]  # (edge_sum, bin, n_nodes)
    heapq.heapify(heap)
    bin_of = np.empty(N, dtype=np.int64)
    slot_of = np.empty(N, dtype=np.int64)
    stash = []
    for n in order_nodes:
        while True:
            s_, b, cnt = heapq.heappop(heap)
            if cnt < WINDOW:
                break
            stash.append((s_, b, cnt))  # full bin, drop
        bin_of[n] = b
        slot_of[n] = cnt
        heapq.heappush(heap, (s_ + deg_ns[n], b, cnt + 1))
    # permuted position of node n
    permpos = bin_of * WINDOW + slot_of  # in [0, 50176)
    # inverse: node at permuted position p (or -1)
    inv = np.full(n_cores * n_local, -1, dtype=np.int64)
    inv[permpos] = np.arange(N)

    # table rows: 0 = zero, 1..NP = permuted slots, NP+1 = zero.
    # row(node n) = permpos[n] + 1
    # LOW view = rows [0, min(HALF, NP+2)); HIGH view = rows [HB, HB+HALF)
    NP = n_cores * n_local
    HB = max(0, NP + 2 - HALF)
    # section 0 restricted to the first AllGather region so phase B's
    # first pass can gather while the second exchange piece is in flight
    lowmax_row = min(HALF, NP + 2, 1 + n_cores * W_SPLITS[1] * WINDOW)
    pad_low = 0
    pad_high = NP + 1 - HB

    # table layout: row 0 = zero; A-region rows [1, 1+nc*ASZ) hold the
    # first A_WIN windows of every shard (early AllGather); B-region rows
    # hold the rest; last row = zero.  rowof(p) maps permuted position.
    p_c = permpos // n_local
    p_off = permpos % n_local
    tabrow_of_pos = np.zeros(N, dtype=np.int64)
    roff = 1
    for r in range(len(W_SPLITS) - 1):
        lo, hi = W_SPLITS[r] * WINDOW, W_SPLITS[r + 1] * WINDOW
        rsz = hi - lo
        m = (p_off >= lo) & (p_off < hi)
        tabrow_of_pos[m] = roff + p_c[m] * rsz + (p_off[m] - lo)
        roff += n_cores * rsz
    rows_all = tabrow_of_pos[src]  # gather row per edge
    dstbin = bin_of[dst]
    dstslot = slot_of[dst]

    # group edges by dst bin
    eorder = np.argsort(dstbin, kind="stable")
    g_rows = rows_all[eorder]
    g_slot = dstslot[eorder]
    g_bin = dstbin[eorder]
    starts = np.searchsorted(g_bin, np.arange(n_bins + 1))

    # per (core, window): split edges into LOW/HIGH with flex edges
    # (rows in [HB, lowmax) fit either view). kw must be uniform across
    # cores, so pick per-window section targets jointly: kw_lo = max_c
    # ceil(lo_only/128), all cores fill LOW to that boundary with flex.
    split = {}
    for b in range(n_bins):
        c, w = divmod(b, w_cnt)
        rows = g_rows[starts[b] : starts[b + 1]]
        rel = g_slot[starts[b] : starts[b + 1]].astype(np.float32)
        is_lo_only = rows < HB
        is_hi_only = rows >= lowmax_row
        is_flex = ~is_lo_only & ~is_hi_only
        split[(c, w)] = (
            (rows[is_lo_only], rel[is_lo_only]),
            (rows[is_hi_only], rel[is_hi_only]),
            (rows[is_flex], rel[is_flex]),
        )
    parts = {}
    kw_lo = np.zeros(w_cnt, dtype=np.int64)
    kw_hi = np.zeros(w_cnt, dtype=np.int64)
    for w in range(w_cnt):
        klo_a = max(
            1, max(-(-len(split[(c, w)][0][0]) // CHUNK) for c in range(n_cores))
        )
        khi_a = max(
            -(
                -(len(split[(c, w)][1][0]) + max(
                    0,
                    len(split[(c, w)][2][0])
                    - (klo_a * CHUNK - len(split[(c, w)][0][0])),
                ))
                // CHUNK
            )
            for c in range(n_cores)
        )
        khi_a = max(1, khi_a)
        # alternative: minimize HIGH first
        khi_b = max(
            1, max(-(-len(split[(c, w)][1][0]) // CHUNK) for c in range(n_cores))
        )
        klo_b = max(
            -(
                -(len(split[(c, w)][0][0]) + max(
                    0,
                    len(split[(c, w)][2][0])
                    - (khi_b * CHUNK - len(split[(c, w)][1][0])),
                ))
                // CHUNK
            )
            for c in range(n_cores)
        )
        klo_b = max(1, klo_b)
        if klo_a + khi_a <= klo_b + khi_b:
            kw_lo[w], kw_hi[w], fill_low = klo_a, khi_a, True
        else:
            kw_lo[w], kw_hi[w], fill_low = klo_b, khi_b, False
        for c in range(n_cores):
            (lo_r, lo_s), (hi_r, hi_s), (fx_r, fx_s) = split[(c, w)]
            if fill_low:
                take = min(len(fx_r), kw_lo[w] * CHUNK - len(lo_r))
            else:
                take = len(fx_r) - min(len(fx_r), kw_hi[w] * CHUNK - len(hi_r))
            parts[(c, w, 0)] = (
                np.concatenate([lo_r, fx_r[:take]]),
                np.concatenate([lo_s, fx_s[:take]]),
            )
            parts[(c, w, 1)] = (
                np.concatenate([hi_r, fx_r[take:]]) - HB,
                np.concatenate([hi_s, fx_s[take:]]),
            )
    T_lo, T_hi = int(kw_lo.sum()), int(kw_hi.sum())
    T = T_lo + T_hi

    # chunk order: per window, LOW chunks then HIGH chunks (PSUM stays
    # open across both sections; one eviction per window).
    chunk_win = []  # (window, first_overall, last_overall, section)
    for w in range(w_cnt):
        kt = int(kw_lo[w] + kw_hi[w])
        k = 0
        for sec, kws in ((0, kw_lo), (1, kw_hi)):
            for _ in range(int(kws[w])):
                chunk_win.append((w, k == 0, k == kt - 1, sec))
                k += 1

    per_core = []
    for c in range(n_cores):
        idx_lin = np.zeros(T * CHUNK, dtype=np.int32)
        dstrel = np.zeros((CHUNK, T), dtype=np.float32)
        t = 0
        for w in range(w_cnt):
            for sec, kws, padrow in ((0, kw_lo, pad_low), (1, kw_hi, pad_high)):
                rows, rel = parts[(c, w, sec)]
                n_e = len(rows)
                n_slots = int(kws[w]) * CHUNK
                buf = np.full(n_slots, padrow, dtype=np.int32)
                buf[:n_e] = rows
                idx_lin[t * CHUNK : t * CHUNK + n_slots] = buf
                rbuf = np.zeros(n_slots, dtype=np.float32)
                rbuf[:n_e] = rel
                dstrel[:, t : t + int(kws[w])] = rbuf.reshape(int(kws[w]), CHUNK).T
                t += int(kws[w])
        assert t == T
        idx16 = idx_lin.astype(np.int16).reshape(T * CHUNK // 16, 16).T  # [16, S]
        idx16 = np.tile(idx16, (8, 1))  # [128, S]

        dinvw = np.zeros((WINDOW, w_cnt), dtype=np.float32)
        for w in range(w_cnt):
            nodes = inv[(c * w_cnt + w) * WINDOW : (c * w_cnt + w + 1) * WINDOW]
            valid = nodes >= 0
            dinvw[valid, w] = dinv[nodes[valid]]
        per_core.append({"idx16": idx16, "dstrel": dstrel, "dinvw": dinvw})

    kw_tot = kw_lo + kw_hi
    t_agsplit = [int(kw_tot[: W_SPLITS[r + 1]].sum()) for r in range(len(W_SPLITS) - 2)]

    return {
        "t_agsplit": t_agsplit,
        "tabrow_of_pos": tabrow_of_pos,
        "n_local": n_local,
        "w_cnt": w_cnt,
        "kw_lo": kw_lo,
        "kw_hi": kw_hi,
        "T_lo": T_lo,
        "T_hi": T_hi,
        "T": T,
        "HB": HB,
        "chunk_win": chunk_win,
        "dinv": dinv,
        "permpos": permpos,
        "inv": inv,
        "per_core": per_core,
    }


# --------------------------------------------------------------------------
# Device kernel builder (one program, SPMD across cores)
# --------------------------------------------------------------------------
def _assign_queues(sizes, w0=0.20):
    """Greedy queue assignment: queue 0's desc-gen blocks the gpsimd engine,
    queues 1-3 run async on their own Q7 pairs - so give q0 a smaller share."""
    weights = [w0, (1.0 - w0) / 3, (1.0 - w0) / 3, (1.0 - w0) / 3]
    load = [0.0, 0.0, 0.0, 0.0]
    out = []
    for sz in sizes:
        q = min(range(4), key=lambda k: (load[k] + sz) / weights[k])
        load[q] += sz
        out.append(q)
    return out


def _build(nc, *, N, n_local, d_in, d_hid, n_cls, pp, n_cores, dt_gat, bias_zero, bias2_zero):
    Relu = mybir.ActivationFunctionType.Relu
    Copy = mybir.ActivationFunctionType.Copy
    T, T_lo = pp["T"], pp["T_lo"]
    w_cnt, HB = pp["w_cnt"], pp["HB"]
    chunk_win = pp["chunk_win"]
    d_rep = REP * n_cls  # 64 cols of f32 -> 256B rows

    xtab = nc.dram_tensor("xtab", [N + 2, d_in], dt_gat, kind="ExternalInput")
    w1 = nc.dram_tensor("w1", [d_in, d_hid], F32, kind="ExternalInput")
    w2 = nc.dram_tensor("w2", [d_hid, n_cls], F32, kind="ExternalInput")
    b1bc = nc.dram_tensor("b1bc", [WINDOW, d_hid], F32, kind="ExternalInput")
    b2bc = nc.dram_tensor("b2bc", [WINDOW, n_cls], F32, kind="ExternalInput")
    iota = nc.dram_tensor("iota", [CHUNK, SBATCH * WINDOW], BF16, kind="ExternalInput")
    ident = nc.dram_tensor("ident", [WINDOW, WINDOW], F32, kind="ExternalInput")
    identb = nc.dram_tensor("identb", [WINDOW, WINDOW], dt_gat, kind="ExternalInput")
    xself = nc.dram_tensor("xself", [n_local, d_in], dt_gat, kind="ExternalInput")
    idx_t = nc.dram_tensor("idx16", [CHUNK, T * 8], I16, kind="ExternalInput")
    dstrel_t = nc.dram_tensor("dstrel", [CHUNK, T], BF16, kind="ExternalInput")
    dinvw_t = nc.dram_tensor("dinvw", [WINDOW, w_cnt], F32, kind="ExternalInput")
    out_t = nc.dram_tensor("out", [WINDOW, w_cnt * n_cls], F32, kind="ExternalOutput")

    h2loc = nc.dram_tensor("h2loc", [n_local, d_rep], BF16)
    h2tab = nc.dram_tensor("h2tab", [N + 2, d_rep], BF16, addr_space="Shared")

    # gather groups: runs of chunks sharing (window, section), split into
    # near-equal pieces of <= GSZ; round-robin across the 4 SWDGE queues.
    groups = []
    t0 = 0
    while t0 < T:
        w0, _, _, sec0 = chunk_win[t0]
        t1 = t0
        while t1 < T and chunk_win[t1][0] == w0 and chunk_win[t1][3] == sec0:
            t1 += 1
        run = t1 - t0
        n_pieces = -(-run // GSZ)
        base = run // n_pieces
        rem = run % n_pieces
        tt0 = t0
        for i in range(n_pieces):
            n = base + (1 if i < rem else 0)
            groups.append((sec0, tt0, n))
            tt0 += n
        t0 = t1
    qa = _assign_queues([g[2] for g in groups])
    groups = [(sec0, tt0, n, q) for (sec0, tt0, n), q in zip(groups, qa)]

    # (row offset into h2tab/xtab, local row range) per AllGather piece
    regions = []
    roff = 1
    for r in range(len(W_SPLITS) - 1):
        lo, hi = W_SPLITS[r] * WINDOW, W_SPLITS[r + 1] * WINDOW
        regions.append((roff, lo, hi))
        roff += n_cores * (hi - lo)
    t_agsplit = pp["t_agsplit"]

    def emit_exchange(r):
        roff_r, lo, hi = regions[r]
        if n_cores > 1:
            nc.gpsimd.collective_compute(
                "AllGather",
                mybir.AluOpType.bypass,
                replica_groups=[list(range(n_cores))],
                ins=[h2loc[lo:hi, :]],
                outs=[h2tab[roff_r : roff_r + n_cores * (hi - lo), :]],
            )
        else:
            nc.sync.dma_start(
                out=h2tab[roff_r : roff_r + (hi - lo), :], in_=h2loc[lo:hi, :]
            )

    def tab_view(tab):
        return [
            tab[0 : min(HALF, N + 2), :],
            tab[HB : min(HB + HALF, N + 2), :],
        ]

    with tile.TileContext(nc) as tc:
        with (
            tc.tile_pool(name="const", bufs=1) as cpool,
            tc.tile_pool(name="gbuf", bufs=12) as gpool,
            tc.tile_pool(name="g2buf", bufs=12) as g2pool,
            tc.tile_pool(name="sbat", bufs=8) as spool,
            tc.tile_pool(name="sbat2", bufs=8) as s2pool,
            tc.tile_pool(name="wtmp", bufs=3) as wpool,
            tc.tile_pool(name="selfp", bufs=3) as selfpool,
            tc.tile_pool(name="psA", bufs=4, space="PSUM") as psA,
            tc.tile_pool(name="psW", bufs=4, space="PSUM") as psW,
        ):
            # ---- constants into SBUF ----
            w1_sb = cpool.tile([d_in, d_hid], F32, tag="w1")
            nc.sync.dma_start(out=w1_sb[:], in_=w1[:])
            w2_sb = cpool.tile([d_hid, n_cls], F32, tag="w2")
            nc.sync.dma_start(out=w2_sb[:], in_=w2[:])
            b1_sb = cpool.tile([WINDOW, d_hid], F32, tag="b1")
            nc.sync.dma_start(out=b1_sb[:], in_=b1bc[:])
            b2_sb = cpool.tile([WINDOW, n_cls], F32, tag="b2")
            nc.sync.dma_start(out=b2_sb[:], in_=b2bc[:])
            iota_sb = cpool.tile([CHUNK, SBATCH * WINDOW], BF16, tag="iota")
            nc.sync.dma_start(out=iota_sb[:], in_=iota[:])
            id_sb = cpool.tile([WINDOW, WINDOW], F32, tag="ident")
            nc.sync.dma_start(out=id_sb[:], in_=ident[:])
            idb_sb = cpool.tile([WINDOW, WINDOW], dt_gat, tag="identb")
            nc.sync.dma_start(out=idb_sb[:], in_=identb[:])
            idx_sb = cpool.tile([CHUNK, T * 8], I16, tag="idx")
            nc.sync.dma_start(out=idx_sb[:], in_=idx_t[:])
            dstrel_sb = cpool.tile([CHUNK, T], BF16, tag="dstrel")
            nc.sync.dma_start(out=dstrel_sb[:], in_=dstrel_t[:])
            dinvw_sb = cpool.tile([WINDOW, w_cnt], F32, tag="dinvw")
            nc.sync.dma_start(out=dinvw_sb[:], in_=dinvw_t[:])
            d2_sb = cpool.tile([WINDOW, w_cnt], F32, tag="dinv2")
            nc.vector.tensor_tensor(
                out=d2_sb[:], in0=dinvw_sb[:], in1=dinvw_sb[:],
                op=mybir.AluOpType.mult,
            )

            zrow = cpool.tile([1, d_rep], BF16, tag="zrow")
            nc.vector.memset(zrow[:], 0.0)
            nc.sync.dma_start(out=h2tab[0:1, :], in_=zrow[:1, :])
            nc.sync.dma_start(out=h2tab[N + 1 : N + 2, :], in_=zrow[:1, :])

            def build_s(pool, t0, n, nm):
                """one-hot S for chunks [t0, t0+n) in one DVE op."""
                s_tile = pool.tile([CHUNK, SBATCH * WINDOW], BF16, tag="s", name=nm)
                rel_b = (
                    dstrel_sb[:, t0 : t0 + n]
                    .rearrange("p (b one) -> p b one", one=1)
                    .to_broadcast([CHUNK, n, WINDOW])
                )
                io_v = iota_sb[:, : n * WINDOW].rearrange("p (b j) -> p b j", j=WINDOW)
                s_v = s_tile[:, : n * WINDOW].rearrange("p (b j) -> p b j", j=WINDOW)
                nc.vector.tensor_tensor(
                    out=s_v, in0=io_v, in1=rel_b, op=mybir.AluOpType.is_equal
                )
                return s_tile

            # =========================== PHASE A ===========================
            psum_of_win = {}
            for sec, t0, n, qn in groups:
                gb = gpool.tile([CHUNK, GSZ, d_in], dt_gat, tag="g", name="gb")
                nc.gpsimd.dma_gather(
                    gb[:, :n, :],
                    tab_view(xtab)[sec],
                    idx_sb[:, t0 * 8 : (t0 + n) * 8],
                    n * CHUNK,
                    n * CHUNK,
                    d_in,
                    single_packet=True,
                    queue_num=qn,
                )
                for bt0 in range(t0, t0 + n, SBATCH):
                    bn = min(SBATCH, t0 + n - bt0)
                    s_tile = build_s(spool, bt0, bn, "sA")
                    for t in range(bt0, bt0 + bn):
                        j = t - bt0
                        w, first, last, _sec = chunk_win[t]
                        if first:
                            psum_of_win[w] = psA.tile(
                                [d_in, WINDOW], F32, tag="agg", name="aggps"
                            )
                            xw = selfpool.tile([WINDOW, d_in], dt_gat, tag="xw")
                            nc.sync.dma_start(
                                out=xw[:],
                                in_=xself[w * WINDOW : (w + 1) * WINDOW, :],
                            )
                            nc.tensor.matmul(
                                out=psum_of_win[w][:],
                                lhsT=xw[:],
                                rhs=idb_sb[:],
                                start=True,
                                stop=False,
                            )
                        nc.tensor.matmul(
                            out=psum_of_win[w][:],
                            lhsT=gb[:, t - t0, :],
                            rhs=s_tile[:, j * WINDOW : (j + 1) * WINDOW],
                            start=False,
                            stop=last,
                        )
                        if not last:
                            continue
                        ps = psum_of_win.pop(w)
                        aggw = wpool.tile([d_in, WINDOW], F32, tag="aggw")
                        nc.scalar.activation(out=aggw[:], in_=ps[:], func=Copy)
                        _window_epilogue_A(
                            nc, w, aggw[:], wpool, psW, w1_sb, w2_sb, b1_sb,
                            dinvw_sb, d2_sb, id_sb, idb_sb, h2loc, n_local,
                            d_in, d_hid, n_cls, d_rep, bias_zero,
                        )
                if t0 + n in t_agsplit:
                    # a window range just closed: exchange its h2 rows now,
                    # overlapping the collective with the remaining gathers
                    emit_exchange(t_agsplit.index(t0 + n))

            # ==== h2 exchange, last piece (earlier pieces were emitted ====
            # ==== mid-phase-A and hide under the remaining gathers) ====
            emit_exchange(len(regions) - 1)

            # =========================== PHASE B ===========================
            # Section-major: pass 0 gathers only rows < AVIEW (the region
            # exchanged by the FIRST AllGather piece), so its gathers flow
            # straight out of phase A while the second piece is in flight;
            # pass 1 (HIGH view) accumulates on top of the pass-0 partials.
            kw_lo_a = pp["kw_lo"]
            kw_hi_a = pp["kw_hi"]
            chunk_base = [0] * (w_cnt + 1)
            for w in range(w_cnt):
                chunk_base[w + 1] = chunk_base[w] + int(kw_lo_a[w] + kw_hi_a[w])
            AVIEW = 1 + n_cores * W_SPLITS[1] * WINDOW
            out2_sb = cpool.tile([WINDOW, w_cnt * n_cls], F32, tag="out2")
            bgroups = []
            for sec in (0, 1):
                for w in range(w_cnt):
                    t0w = chunk_base[w] + (0 if sec == 0 else int(kw_lo_a[w]))
                    run = int(kw_lo_a[w]) if sec == 0 else int(kw_hi_a[w])
                    n_pieces = -(-run // GSZ)
                    gbase = run // n_pieces
                    grem = run % n_pieces
                    tt0 = t0w
                    for i in range(n_pieces):
                        n = gbase + (1 if i < grem else 0)
                        bgroups.append((sec, w, tt0, n))
                        tt0 += n
            qb = _assign_queues([g[3] for g in bgroups])
            bgroups = [(sec, w, tt0, n, q) for (sec, w, tt0, n), q in zip(bgroups, qb)]
            psum_of_win = {}
            for sec, w, t0, n, qn in bgroups:
                g2 = g2pool.tile([CHUNK, GSZ, d_rep], BF16, tag="g2", name="g2b")
                view = h2tab[0:AVIEW, :] if sec == 0 else tab_view(h2tab)[1]
                nc.gpsimd.dma_gather(
                    g2[:, :n, :],
                    view,
                    idx_sb[:, t0 * 8 : (t0 + n) * 8],
                    n * CHUNK,
                    n * CHUNK,
                    d_rep,
                    single_packet=True,
                    queue_num=qn,
                )
                for bt0 in range(t0, t0 + n, SBATCH):
                    bn = min(SBATCH, t0 + n - bt0)
                    s_tile = build_s(s2pool, bt0, bn, "sB")
                    for t in range(bt0, bt0 + bn):
                        j = t - bt0
                        sec0_first = sec == 0 and t == chunk_base[w]
                        sec0_last = (
                            sec == 0 and t == chunk_base[w] + int(kw_lo_a[w]) - 1
                        )
                        sec1_first = (
                            sec == 1 and t == chunk_base[w] + int(kw_lo_a[w])
                        )
                        sec1_last = sec == 1 and t == chunk_base[w + 1] - 1
                        if sec0_first:
                            psum_of_win[w] = psA.tile(
                                [WINDOW, n_cls], F32, tag="agg", name="agg2ps"
                            )
                            h2w = selfpool.tile([WINDOW, d_rep], BF16, tag="h2w")
                            nc.sync.dma_start(
                                out=h2w[:],
                                in_=h2loc[w * WINDOW : (w + 1) * WINDOW, :],
                            )
                            nc.tensor.matmul(
                                out=psum_of_win[w][:],
                                lhsT=idb_sb[:],
                                rhs=h2w[:, :n_cls],
                                start=True,
                                stop=False,
                            )
                        elif sec1_first:
                            psum_of_win[w] = psA.tile(
                                [WINDOW, n_cls], F32, tag="agg", name="agg2ps"
                            )
                        nc.tensor.matmul(
                            out=psum_of_win[w][:],
                            lhsT=s_tile[:, j * WINDOW : (j + 1) * WINDOW],
                            rhs=g2[:, t - t0, :n_cls],
                            start=sec1_first,
                            stop=sec0_last or sec1_last,
                        )
                        osl = out2_sb[:, w * n_cls : (w + 1) * n_cls]
                        if sec0_last:
                            ps = psum_of_win.pop(w)
                            nc.scalar.activation(out=osl, in_=ps[:], func=Copy)
                        elif sec1_last:
                            ps = psum_of_win.pop(w)
                            ob = wpool.tile([WINDOW, n_cls], F32, tag="ob")
                            nc.vector.tensor_tensor(
                                out=ob[:], in0=ps[:], in1=osl,
                                op=mybir.AluOpType.add,
                            )
                            if bias2_zero:
                                nc.vector.tensor_scalar(
                                    out=osl,
                                    in0=ob[:],
                                    scalar1=dinvw_sb[:, w : w + 1],
                                    scalar2=None,
                                    op0=mybir.AluOpType.mult,
                                )
                            else:
                                ob2 = wpool.tile([WINDOW, n_cls], F32, tag="ob2")
                                nc.vector.tensor_scalar(
                                    out=ob2[:],
                                    in0=ob[:],
                                    scalar1=dinvw_sb[:, w : w + 1],
                                    scalar2=None,
                                    op0=mybir.AluOpType.mult,
                                )
                                nc.vector.tensor_tensor(
                                    out=osl, in0=ob2[:], in1=b2_sb[:],
                                    op=mybir.AluOpType.add,
                                )
            nc.sync.dma_start(out=out_t[:], in_=out2_sb[:])

    nc.compile()
    return nc


def _window_epilogue_A(
    nc, w, aggT, wpool, psW, w1_sb, w2_sb, b1_sb, dinvw_sb, d2_sb, id_sb,
    idb_sb, h2loc, n_local, d_in, d_hid, n_cls, d_rep, bias_zero,
):
    """aggT [d_in, WINDOW] in SBUF -> replicated h2 rows in DRAM."""
    Relu = mybir.ActivationFunctionType.Relu
    Copy = mybir.ActivationFunctionType.Copy

    # h1 [dst, hid] = aggT.T @ W1
    h1_ps = psW.tile([WINDOW, d_hid], F32, tag="wps", name="h1_ps")
    nc.tensor.matmul(out=h1_ps[:], lhsT=aggT, rhs=w1_sb[:], start=True, stop=True)
    if bias_zero:
        # b1 == 0: relu(dinv*x) = dinv*relu(x); relu straight off PSUM and
        # fold both dinv factors into the final replicate-scale (dinv^2).
        r3_sb = wpool.tile([WINDOW, d_hid], F32, tag="r3")
        nc.scalar.activation(out=r3_sb[:], in_=h1_ps[:], func=Relu)
        fin_scale = d2_sb
    else:
        r_sb = wpool.tile([WINDOW, d_hid], F32, tag="r")
        nc.vector.tensor_scalar(
            out=r_sb[:],
            in0=h1_ps[:],
            scalar1=dinvw_sb[:, w : w + 1],
            scalar2=None,
            op0=mybir.AluOpType.mult,
        )
        r2_sb = wpool.tile([WINDOW, d_hid], F32, tag="r2")
        nc.vector.tensor_tensor(
            out=r2_sb[:], in0=r_sb[:], in1=b1_sb[:], op=mybir.AluOpType.add
        )
        r3_sb = wpool.tile([WINDOW, d_hid], F32, tag="r3")
        nc.scalar.activation(out=r3_sb[:], in_=r2_sb[:], func=Relu)
        fin_scale = dinvw_sb
    # transpose -> [hid, dst]
    rT_ps = psW.tile([d_hid, WINDOW], F32, tag="wps", name="rT_ps")
    nc.tensor.transpose(out=rT_ps[:], in_=r3_sb[:], identity=id_sb[:])
    rT_sb = wpool.tile([d_hid, WINDOW], F32, tag="rTs")
    nc.scalar.activation(out=rT_sb[:], in_=rT_ps[:], func=Copy)
    # h2 [dst, n_cls] = rT.T @ W2; scale; replicate REP x
    h2_ps = psW.tile([WINDOW, n_cls], F32, tag="wps", name="h2_ps")
    nc.tensor.matmul(out=h2_ps[:], lhsT=rT_sb[:], rhs=w2_sb[:], start=True, stop=True)
    h2_sb = wpool.tile([WINDOW, d_rep], BF16, tag="h2s")
    nc.vector.tensor_scalar(
        out=h2_sb[:].rearrange("p (r c) -> p r c", c=n_cls),
        in0=h2_ps[:]
        .rearrange("p (one c) -> p one c", one=1)
        .to_broadcast([WINDOW, REP, n_cls]),
        scalar1=fin_scale[:, w : w + 1],
        scalar2=None,
        op0=mybir.AluOpType.mult,
    )
    nrows = min(WINDOW, n_local - w * WINDOW)
    nc.sync.dma_start(
        out=h2loc[w * WINDOW : w * WINDOW + nrows, :], in_=h2_sb[:nrows, :]
    )


# --------------------------------------------------------------------------
# Entry point
# --------------------------------------------------------------------------
def _make_inputs(x, W1, b1, W2, b2, pp, dt_np, n_cores):
    N, d_in = x.shape
    NP = n_cores * pp["n_local"]
    W1 = np.asarray(W1, np.float32)
    b1 = np.asarray(b1, np.float32)
    W2 = np.asarray(W2, np.float32)
    b2 = np.asarray(b2, np.float32)
    d_hid = W1.shape[1]
    n_cls = W2.shape[1]
    xp = x * pp["dinv"][:, None]
    xtab = np.zeros((NP + 2, d_in), np.float32)
    xtab[pp["tabrow_of_pos"]] = xp
    xtab = xtab.astype(dt_np)
    iota_arr = np.broadcast_to(
        np.tile(np.arange(WINDOW, dtype=np.float32), SBATCH),
        (CHUNK, SBATCH * WINDOW),
    ).astype(np.dtype("bfloat16"))
    shared = {
        "xtab": xtab,
        "w1": W1,
        "w2": W2,
        "b1bc": np.broadcast_to(b1, (WINDOW, d_hid)).astype(np.float32).copy(),
        "b2bc": np.broadcast_to(b2, (WINDOW, n_cls)).astype(np.float32).copy(),
        "iota": iota_arr,
        "ident": np.eye(WINDOW, dtype=np.float32),
        "identb": np.eye(WINDOW, dtype=np.float32).astype(dt_np),
    }
    n_local = pp["n_local"]
    in_maps = []
    for c, pc in enumerate(pp["per_core"]):
        m = dict(shared)
        xs = np.zeros((n_local, d_in), np.float32)
        sl = pp["inv"][c * n_local : (c + 1) * n_local]
        xs[sl >= 0] = xp[sl[sl >= 0]]
        m["xself"] = xs.astype(dt_np)
        m["idx16"] = pc["idx16"]
        m["dstrel"] = pc["dstrel"].astype(np.dtype("bfloat16"))
        m["dinvw"] = pc["dinvw"]
        in_maps.append(m)
    return in_maps


def _run(x, edge_index, W1, b1, W2, b2, n_cores, trace=False):
    x = np.asarray(x, dtype=np.float32)
    N, d_in = x.shape
    d_hid = np.asarray(W1).shape[1]
    n_cls = np.asarray(W2).shape[1]
    assert d_in == 128 and d_hid == 128

    pp = _preprocess(x, edge_index, n_cores)
    dt_gat = BF16 if GATHER_BF16 else F32
    np_gat = np.dtype("bfloat16") if GATHER_BF16 else np.dtype("float32")

    nc = bacc.Bacc("TRN2", target_bir_lowering=False, debug=False, num_swdge_queues=4)
    _build(
        nc,
        N=n_cores * pp["n_local"],
        n_local=pp["n_local"],
        d_in=d_in,
        d_hid=d_hid,
        n_cls=n_cls,
        pp=pp,
        n_cores=n_cores,
        dt_gat=dt_gat,
        bias_zero=bool(np.all(np.asarray(b1) == 0)),
        bias2_zero=bool(np.all(np.asarray(b2) == 0)),
    )

    import ml_dtypes  # noqa

    in_maps = _make_inputs(x, W1, b1, W2, b2, pp, np_gat, n_cores)
    res = run_bass_kernel_spmd(nc, in_maps, list(range(n_cores)), trace=trace)
    w_cnt = pp["w_cnt"]
    outs = [
        res.results[c]["out"]
        .reshape(WINDOW, w_cnt, 2)
        .transpose(1, 0, 2)
        .reshape(pp["n_local"], 2)
        for c in range(n_cores)
    ]
    full = np.concatenate(outs, axis=0)[pp["permpos"]]
    return full.astype(np.float32), res


def kernel(x, edge_index, W1, b1, W2, b2):
    out, _ = _run(x, edge_index, W1, b1, W2, b2, N_CORES)
    return out



# revision 34
# speedup vs baseline: 1.0799x; 1.0033x over previous
"""GCN 2-layer (PyG GCNConv x2 + ReLU) Bass kernel for Trainium2, 8-core SPMD.

Strategy:
  - Host: compute symmetric normalization dinv = deg^-1/2 (self-loop incl.),
    fold dinv[src] into a prescaled gather table (x * dinv).  Dst nodes are
    RELABELED via degree-balanced LPT binning into 8 cores x 49 windows of
    128 nodes, so every (core, window) has a near-equal edge count and chunk
    padding is <1%.  Per core, edges sorted by dst window and packed into
    128-edge chunks (one matmul each).  dma_gather uses int16 indices, so
    the node table is addressed via two 32768-row views (LOW/HIGH); srcs in
    the overlap may go to either section and are assigned to fill chunk
    boundaries exactly.  Each window's LOW and HIGH chunks run back to
    back so its PSUM accumulation closes in one pass (single eviction,
    epilogues spread across the phase).  Biases are folded away when zero
    (relu(dinv*x) = dinv*relu(x); dinv^2 in the final replicate-scale).
    Self-loop edges are NOT gathered: the permuted
    table makes each window's own rows contiguous, so they stream in with a
    plain DMA and join the accumulation as one identity-operand matmul.
  - Device per core:
      Gathers are issued round-robin on the 4 SWDGE queues (num_swdge_
      queues=4).  Descriptor generation for queues 1-3 runs on their own Q7
      core pairs concurrently under queue 0's engine-blocking gather,
      cutting effective desc-gen from 8.4 ns/idx to ~2.6 ns/idx.
      Phase A (layer 1): dma_gather source rows of the prescaled x-table ->
        G [128e, d_in]; build one-hot S [128e, 128dst] on DVE (bf16 iota ==
        dst_rel); PE matmul accumulates G.T @ S into PSUM [d_in, 128dst]
        per window.  Per window: x W1 (PE), scale by dinv[dst], +b1, ReLU;
        transpose (PE); x W2; scale by dinv[dst]; replicate 64x -> 256B
        rows of the h2 table, DMA out.
      AllGather h2 shards -> full replicated table.
      Phase B (layer 2): same chunk structure; gather h2 rows, matmul
        S.T @ G2[:, :2] accumulated per window; scale by dinv[dst], +b2.
    Output rows are in permuted order; the host scatters them back.
"""

import numpy as np

import concourse.bass as bass
import concourse.mybir as mybir
import concourse.tile as tile
from concourse import bacc
from concourse.bass_utils import run_bass_kernel_spmd

F32 = mybir.dt.float32
BF16 = mybir.dt.bfloat16
I16 = mybir.dt.int16

N_CORES = 8
WINDOW = 128  # dst nodes per PSUM accumulation window
CHUNK = 128  # edges per matmul chunk
GSZ = 8  # max chunks per dma_gather instruction (1024 idxs, single-packet)
SBATCH = 8  # chunks per S-build DVE op
HALF = 32768  # int16 index range
REP = 64  # h2 replication (64x2 bf16 cols -> 256B rows)
W_SPLITS = (0, 25, 49)  # h2 AllGather pieces: window ranges, exchanged as each closes
GATHER_BF16 = True  # layer-1 gather table + chunk matmuls in bf16


# --------------------------------------------------------------------------
# Host preprocessing
# --------------------------------------------------------------------------
def _preprocess(x, edge_index, n_cores):
    N = x.shape[0]
    src = np.asarray(edge_index[0], dtype=np.int64)
    dst = np.asarray(edge_index[1], dtype=np.int64)
    # degree includes the self-loop (handled separately on-device)
    deg = (np.bincount(dst, minlength=N) + 1).astype(np.float64)
    dinv = np.where(deg > 0, 1.0 / np.sqrt(deg), 0.0).astype(np.float32)

    # ---- degree-balanced dst binning: 392 bins (8 cores x 49 windows), ----
    # ---- capacity 128 nodes each; LPT greedy on in-degree ----
    import heapq

    w_cnt = 49
    n_bins = n_cores * w_cnt
    n_local = w_cnt * WINDOW  # 6272 (padded shard; empty slots get dinv 0)
    deg_ns = np.bincount(dst, minlength=N).astype(np.float64)
    order_nodes = np.argsort(-deg_ns, kind="stable")
    heap = [(0.0, b, 0) for b in range(n_bins)]  # (edge_sum, bin, n_nodes)
    heapq.heapify(heap)
    bin_of = np.empty(N, dtype=np.int64)
    slot_of = np.empty(N, dtype=np.int64)
    stash = []
    for n in order_nodes:
        while True:
            s_, b, cnt = heapq.heappop(heap)
            if cnt < WINDOW:
                break
            stash.append((s_, b, cnt))  # full bin, drop
        bin_of[n] = b
        slot_of[n] = cnt
        heapq.heappush(heap, (s_ + deg_ns[n], b, cnt + 1))
    # permuted position of node n
    permpos = bin_of * WINDOW + slot_of  # in [0, 50176)
    # inverse: node at permuted position p (or -1)
    inv = np.full(n_cores * n_local, -1, dtype=np.int64)
    inv[permpos] = np.arange(N)

    # table rows: 0 = zero, 1..NP = permuted slots, NP+1 = zero.
    # row(node n) = permpos[n] + 1
    # LOW view = rows [0, min(HALF, NP+2)); HIGH view = rows [HB, HB+HALF)
    NP = n_cores * n_local
    HB = max(0, NP + 2 - HALF)
    # section 0 restricted to the first AllGather region so phase B's
    # first pass can gather while the second exchange piece is in flight
    lowmax_row = min(HALF, NP + 2, 1 + n_cores * W_SPLITS[1] * WINDOW)
    pad_low = 0
    pad_high = NP + 1 - HB

    # table layout: row 0 = zero; A-region rows [1, 1+nc*ASZ) hold the
    # first A_WIN windows of every shard (early AllGather); B-region rows
    # hold the rest; last row = zero.  rowof(p) maps permuted position.
    p_c = permpos // n_local
    p_off = permpos % n_local
    tabrow_of_pos = np.zeros(N, dtype=np.int64)
    roff = 1
    for r in range(len(W_SPLITS) - 1):
        lo, hi = W_SPLITS[r] * WINDOW, W_SPLITS[r + 1] * WINDOW
        rsz = hi - lo
        m = (p_off >= lo) & (p_off < hi)
        tabrow_of_pos[m] = roff + p_c[m] * rsz + (p_off[m] - lo)
        roff += n_cores * rsz
    rows_all = tabrow_of_pos[src]  # gather row per edge
    dstbin = bin_of[dst]
    dstslot = slot_of[dst]

    # group edges by dst bin
    eorder = np.argsort(dstbin, kind="stable")
    g_rows = rows_all[eorder]
    g_slot = dstslot[eorder]
    g_bin = dstbin[eorder]
    starts = np.searchsorted(g_bin, np.arange(n_bins + 1))

    # per (core, window): split edges into LOW/HIGH with flex edges
    # (rows in [HB, lowmax) fit either view). kw must be uniform across
    # cores, so pick per-window section targets jointly: kw_lo = max_c
    # ceil(lo_only/128), all cores fill LOW to that boundary with flex.
    split = {}
    for b in range(n_bins):
        c, w = divmod(b, w_cnt)
        rows = g_rows[starts[b] : starts[b + 1]]
        rel = g_slot[starts[b] : starts[b + 1]].astype(np.float32)
        is_lo_only = rows < HB
        is_hi_only = rows >= lowmax_row
        is_flex = ~is_lo_only & ~is_hi_only
        split[(c, w)] = (
            (rows[is_lo_only], rel[is_lo_only]),
            (rows[is_hi_only], rel[is_hi_only]),
            (rows[is_flex], rel[is_flex]),
        )
    parts = {}
    kw_lo = np.zeros(w_cnt, dtype=np.int64)
    kw_hi = np.zeros(w_cnt, dtype=np.int64)
    for w in range(w_cnt):
        klo_a = max(
            1, max(-(-len(split[(c, w)][0][0]) // CHUNK) for c in range(n_cores))
        )
        khi_a = max(
            -(
                -(len(split[(c, w)][1][0]) + max(
                    0,
                    len(split[(c, w)][2][0])
                    - (klo_a * CHUNK - len(split[(c, w)][0][0])),
                ))
                // CHUNK
            )
            for c in range(n_cores)
        )
        khi_a = max(1, khi_a)
        # alternative: minimize HIGH first
        khi_b = max(
            1, max(-(-len(split[(c, w)][1][0]) // CHUNK) for c in range(n_cores))
        )
        klo_b = max(
            -(
                -(len(split[(c, w)][0][0]) + max(
                    0,
                    len(split[(c, w)][2][0])
                    - (khi_b * CHUNK - len(split[(c, w)][1][0])),
                ))
                // CHUNK
            )
            for c in range(n_cores)
        )
        klo_b = max(1, klo_b)
        if klo_a + khi_a <= klo_b + khi_b:
            kw_lo[w], kw_hi[w], fill_low = klo_a, khi_a, True
        else:
            kw_lo[w], kw_hi[w], fill_low = klo_b, khi_b, False
        for c in range(n_cores):
            (lo_r, lo_s), (hi_r, hi_s), (fx_r, fx_s) = split[(c, w)]
            if fill_low:
                take = min(len(fx_r), kw_lo[w] * CHUNK - len(lo_r))
            else:
                take = len(fx_r) - min(len(fx_r), kw_hi[w] * CHUNK - len(hi_r))
            parts[(c, w, 0)] = (
                np.concatenate([lo_r, fx_r[:take]]),
                np.concatenate([lo_s, fx_s[:take]]),
            )
            parts[(c, w, 1)] = (
                np.concatenate([hi_r, fx_r[take:]]) - HB,
                np.concatenate([hi_s, fx_s[take:]]),
            )
    T_lo, T_hi = int(kw_lo.sum()), int(kw_hi.sum())
    T = T_lo + T_hi

    # chunk order: per window, LOW chunks then HIGH chunks (PSUM stays
    # open across both sections; one eviction per window).
    chunk_win = []  # (window, first_overall, last_overall, section)
    for w in range(w_cnt):
        kt = int(kw_lo[w] + kw_hi[w])
        k = 0
        for sec, kws in ((0, kw_lo), (1, kw_hi)):
            for _ in range(int(kws[w])):
                chunk_win.append((w, k == 0, k == kt - 1, sec))
                k += 1

    per_core = []
    for c in range(n_cores):
        idx_lin = np.zeros(T * CHUNK, dtype=np.int32)
        dstrel = np.zeros((CHUNK, T), dtype=np.float32)
        t = 0
        for w in range(w_cnt):
            for sec, kws, padrow in ((0, kw_lo, pad_low), (1, kw_hi, pad_high)):
                rows, rel = parts[(c, w, sec)]
                n_e = len(rows)
                n_slots = int(kws[w]) * CHUNK
                buf = np.full(n_slots, padrow, dtype=np.int32)
                buf[:n_e] = rows
                idx_lin[t * CHUNK : t * CHUNK + n_slots] = buf
                rbuf = np.zeros(n_slots, dtype=np.float32)
                rbuf[:n_e] = rel
                dstrel[:, t : t + int(kws[w])] = rbuf.reshape(int(kws[w]), CHUNK).T
                t += int(kws[w])
        assert t == T
        idx16 = idx_lin.astype(np.int16).reshape(T * CHUNK // 16, 16).T  # [16, S]
        idx16 = np.tile(idx16, (8, 1))  # [128, S]

        dinvw = np.zeros((WINDOW, w_cnt), dtype=np.float32)
        for w in range(w_cnt):
            nodes = inv[(c * w_cnt + w) * WINDOW : (c * w_cnt + w + 1) * WINDOW]
            valid = nodes >= 0
            dinvw[valid, w] = dinv[nodes[valid]]
        per_core.append({"idx16": idx16, "dstrel": dstrel, "dinvw": dinvw})

    kw_tot = kw_lo + kw_hi
    t_agsplit = [int(kw_tot[: W_SPLITS[r + 1]].sum()) for r in range(len(W_SPLITS) - 2)]

    return {
        "t_agsplit": t_agsplit,
        "tabrow_of_pos": tabrow_of_pos,
        "n_local": n_local,
        "w_cnt": w_cnt,
        "kw_lo": kw_lo,
        "kw_hi": kw_hi,
        "T_lo": T_lo,
        "T_hi": T_hi,
        "T": T,
        "HB": HB,
        "chunk_win": chunk_win,
        "dinv": dinv,
        "permpos": permpos,
        "inv": inv,
        "per_core": per_core,
    }


# --------------------------------------------------------------------------
# Device kernel builder (one program, SPMD across cores)
# --------------------------------------------------------------------------
def _assign_queues(sizes, w0=0.20):
    """Greedy queue assignment: queue 0's desc-gen blocks the gpsimd engine,
    queues 1-3 run async on their own Q7 pairs - so give q0 a smaller share."""
    weights = [w0, (1.0 - w0) / 3, (1.0 - w0) / 3, (1.0 - w0) / 3]
    load = [0.0, 0.0, 0.0, 0.0]
    out = []
    for sz in sizes:
        q = min(range(4), key=lambda k: (load[k] + sz) / weights[k])
        load[q] += sz
        out.append(q)
    return out


def _build(nc, *, N, n_local, d_in, d_hid, n_cls, pp, n_cores, dt_gat, bias_zero, bias2_zero):
    Relu = mybir.ActivationFunctionType.Relu
    Copy = mybir.ActivationFunctionType.Copy
    T, T_lo = pp["T"], pp["T_lo"]
    w_cnt, HB = pp["w_cnt"], pp["HB"]
    chunk_win = pp["chunk_win"]
    d_rep = REP * n_cls  # 64 cols of f32 -> 256B rows

    xtab = nc.dram_tensor("xtab", [N + 2, d_in], dt_gat, kind="ExternalInput")
    w1 = nc.dram_tensor("w1", [d_in, d_hid], F32, kind="ExternalInput")
    w2 = nc.dram_tensor("w2", [d_hid, n_cls], F32, kind="ExternalInput")
    b1bc = nc.dram_tensor("b1bc", [WINDOW, d_hid], F32, kind="ExternalInput")
    b2bc = nc.dram_tensor("b2bc", [WINDOW, n_cls], F32, kind="ExternalInput")
    iota = nc.dram_tensor("iota", [CHUNK, SBATCH * WINDOW], BF16, kind="ExternalInput")
    ident = nc.dram_tensor("ident", [WINDOW, WINDOW], F32, kind="ExternalInput")
    identb = nc.dram_tensor("identb", [WINDOW, WINDOW], dt_gat, kind="ExternalInput")
    xself = nc.dram_tensor("xself", [n_local, d_in], dt_gat, kind="ExternalInput")
    idx_t = nc.dram_tensor("idx16", [CHUNK, T * 8], I16, kind="ExternalInput")
    dstrel_t = nc.dram_tensor("dstrel", [CHUNK, T], BF16, kind="ExternalInput")
    dinvw_t = nc.dram_tensor("dinvw", [WINDOW, w_cnt], F32, kind="ExternalInput")
    out_t = nc.dram_tensor("out", [WINDOW, w_cnt * n_cls], F32, kind="ExternalOutput")

    h2loc = nc.dram_tensor("h2loc", [n_local, d_rep], BF16)
    h2tab = nc.dram_tensor("h2tab", [N + 2, d_rep], BF16, addr_space="Shared")

    # gather groups: runs of chunks sharing (window, section), split into
    # near-equal pieces of <= GSZ; round-robin across the 4 SWDGE queues.
    groups = []
    t0 = 0
    while t0 < T:
        w0, _, _, sec0 = chunk_win[t0]
        t1 = t0
        while t1 < T and chunk_win[t1][0] == w0 and chunk_win[t1][3] == sec0:
            t1 += 1
        run = t1 - t0
        n_pieces = -(-run // GSZ)
        base = run // n_pieces
        rem = run % n_pieces
        tt0 = t0
        for i in range(n_pieces):
            n = base + (1 if i < rem else 0)
            groups.append((sec0, tt0, n))
            tt0 += n
        t0 = t1
    qa = _assign_queues([g[2] for g in groups])
    groups = [(sec0, tt0, n, q) for (sec0, tt0, n), q in zip(groups, qa)]

    # (row offset into h2tab/xtab, local row range) per AllGather piece
    regions = []
    roff = 1
    for r in range(len(W_SPLITS) - 1):
        lo, hi = W_SPLITS[r] * WINDOW, W_SPLITS[r + 1] * WINDOW
        regions.append((roff, lo, hi))
        roff += n_cores * (hi - lo)
    t_agsplit = pp["t_agsplit"]

    def emit_exchange(r):
        roff_r, lo, hi = regions[r]
        if n_cores > 1:
            nc.gpsimd.collective_compute(
                "AllGather",
                mybir.AluOpType.bypass,
                replica_groups=[list(range(n_cores))],
                ins=[h2loc[lo:hi, :]],
                outs=[h2tab[roff_r : roff_r + n_cores * (hi - lo), :]],
            )
        else:
            nc.sync.dma_start(
                out=h2tab[roff_r : roff_r + (hi - lo), :], in_=h2loc[lo:hi, :]
            )

    def tab_view(tab):
        return [
            tab[0 : min(HALF, N + 2), :],
            tab[HB : min(HB + HALF, N + 2), :],
        ]

    with tile.TileContext(nc) as tc:
        with (
            tc.tile_pool(name="const", bufs=1) as cpool,
            tc.tile_pool(name="gbuf", bufs=12) as gpool,
            tc.tile_pool(name="g2buf", bufs=12) as g2pool,
            tc.tile_pool(name="sbat", bufs=8) as spool,
            tc.tile_pool(name="sbat2", bufs=8) as s2pool,
            tc.tile_pool(name="wtmp", bufs=3) as wpool,
            tc.tile_pool(name="selfp", bufs=3) as selfpool,
            tc.tile_pool(name="psA", bufs=5, space="PSUM") as psA,
            tc.tile_pool(name="psW", bufs=3, space="PSUM") as psW,
        ):
            # ---- constants into SBUF ----
            w1_sb = cpool.tile([d_in, d_hid], F32, tag="w1")
            nc.sync.dma_start(out=w1_sb[:], in_=w1[:])
            w2_sb = cpool.tile([d_hid, n_cls], F32, tag="w2")
            nc.sync.dma_start(out=w2_sb[:], in_=w2[:])
            b1_sb = cpool.tile([WINDOW, d_hid], F32, tag="b1")
            nc.sync.dma_start(out=b1_sb[:], in_=b1bc[:])
            b2_sb = cpool.tile([WINDOW, n_cls], F32, tag="b2")
            nc.sync.dma_start(out=b2_sb[:], in_=b2bc[:])
            iota_sb = cpool.tile([CHUNK, SBATCH * WINDOW], BF16, tag="iota")
            nc.sync.dma_start(out=iota_sb[:], in_=iota[:])
            id_sb = cpool.tile([WINDOW, WINDOW], F32, tag="ident")
            nc.sync.dma_start(out=id_sb[:], in_=ident[:])
            idb_sb = cpool.tile([WINDOW, WINDOW], dt_gat, tag="identb")
            nc.sync.dma_start(out=idb_sb[:], in_=identb[:])
            idx_sb = cpool.tile([CHUNK, T * 8], I16, tag="idx")
            nc.sync.dma_start(out=idx_sb[:], in_=idx_t[:])
            dstrel_sb = cpool.tile([CHUNK, T], BF16, tag="dstrel")
            nc.sync.dma_start(out=dstrel_sb[:], in_=dstrel_t[:])
            dinvw_sb = cpool.tile([WINDOW, w_cnt], F32, tag="dinvw")
            nc.sync.dma_start(out=dinvw_sb[:], in_=dinvw_t[:])
            d2_sb = cpool.tile([WINDOW, w_cnt], F32, tag="dinv2")
            nc.vector.tensor_tensor(
                out=d2_sb[:], in0=dinvw_sb[:], in1=dinvw_sb[:],
                op=mybir.AluOpType.mult,
            )

            zrow = cpool.tile([1, d_rep], BF16, tag="zrow")
            nc.vector.memset(zrow[:], 0.0)
            nc.sync.dma_start(out=h2tab[0:1, :], in_=zrow[:1, :])
            nc.sync.dma_start(out=h2tab[N + 1 : N + 2, :], in_=zrow[:1, :])

            def build_s(pool, t0, n, nm):
                """one-hot S for chunks [t0, t0+n) in one DVE op."""
                s_tile = pool.tile([CHUNK, SBATCH * WINDOW], BF16, tag="s", name=nm)
                rel_b = (
                    dstrel_sb[:, t0 : t0 + n]
                    .rearrange("p (b one) -> p b one", one=1)
                    .to_broadcast([CHUNK, n, WINDOW])
                )
                io_v = iota_sb[:, : n * WINDOW].rearrange("p (b j) -> p b j", j=WINDOW)
                s_v = s_tile[:, : n * WINDOW].rearrange("p (b j) -> p b j", j=WINDOW)
                nc.vector.tensor_tensor(
                    out=s_v, in0=io_v, in1=rel_b, op=mybir.AluOpType.is_equal
                )
                return s_tile

            # =========================== PHASE A ===========================
            psum_of_win = {}
            for sec, t0, n, qn in groups:
                gb = gpool.tile([CHUNK, GSZ, d_in], dt_gat, tag="g", name="gb")
                nc.gpsimd.dma_gather(
                    gb[:, :n, :],
                    tab_view(xtab)[sec],
                    idx_sb[:, t0 * 8 : (t0 + n) * 8],
                    n * CHUNK,
                    n * CHUNK,
                    d_in,
                    single_packet=True,
                    queue_num=qn,
                )
                for bt0 in range(t0, t0 + n, SBATCH):
                    bn = min(SBATCH, t0 + n - bt0)
                    s_tile = build_s(spool, bt0, bn, "sA")
                    for t in range(bt0, bt0 + bn):
                        j = t - bt0
                        w, first, last, _sec = chunk_win[t]
                        if first:
                            psum_of_win[w] = psA.tile(
                                [d_in, WINDOW], F32, tag="agg", name="aggps"
                            )
                            xw = selfpool.tile([WINDOW, d_in], dt_gat, tag="xw")
                            nc.sync.dma_start(
                                out=xw[:],
                                in_=xself[w * WINDOW : (w + 1) * WINDOW, :],
                            )
                            nc.tensor.matmul(
                                out=psum_of_win[w][:],
                                lhsT=xw[:],
                                rhs=idb_sb[:],
                                start=True,
                                stop=False,
                            )
                        nc.tensor.matmul(
                            out=psum_of_win[w][:],
                            lhsT=gb[:, t - t0, :],
                            rhs=s_tile[:, j * WINDOW : (j + 1) * WINDOW],
                            start=False,
                            stop=last,
                        )
                        if not last:
                            continue
                        ps = psum_of_win.pop(w)
                        aggw = wpool.tile([d_in, WINDOW], F32, tag="aggw")
                        nc.scalar.activation(out=aggw[:], in_=ps[:], func=Copy)
                        _window_epilogue_A(
                            nc, w, aggw[:], wpool, psW, w1_sb, w2_sb, b1_sb,
                            dinvw_sb, d2_sb, id_sb, idb_sb, h2loc, n_local,
                            d_in, d_hid, n_cls, d_rep, bias_zero,
                        )
                if t0 + n in t_agsplit:
                    # a window range just closed: exchange its h2 rows now,
                    # overlapping the collective with the remaining gathers
                    emit_exchange(t_agsplit.index(t0 + n))

            # ==== h2 exchange, last piece (earlier pieces were emitted ====
            # ==== mid-phase-A and hide under the remaining gathers) ====
            emit_exchange(len(regions) - 1)

            # =========================== PHASE B ===========================
            # Section-major: pass 0 gathers only rows < AVIEW (the region
            # exchanged by the FIRST AllGather piece), so its gathers flow
            # straight out of phase A while the second piece is in flight;
            # pass 1 (HIGH view) accumulates on top of the pass-0 partials.
            kw_lo_a = pp["kw_lo"]
            kw_hi_a = pp["kw_hi"]
            chunk_base = [0] * (w_cnt + 1)
            for w in range(w_cnt):
                chunk_base[w + 1] = chunk_base[w] + int(kw_lo_a[w] + kw_hi_a[w])
            AVIEW = 1 + n_cores * W_SPLITS[1] * WINDOW
            out2_sb = cpool.tile([WINDOW, w_cnt * n_cls], F32, tag="out2")
            bgroups = []
            for sec in (0, 1):
                for w in range(w_cnt):
                    t0w = chunk_base[w] + (0 if sec == 0 else int(kw_lo_a[w]))
                    run = int(kw_lo_a[w]) if sec == 0 else int(kw_hi_a[w])
                    n_pieces = -(-run // GSZ)
                    gbase = run // n_pieces
                    grem = run % n_pieces
                    tt0 = t0w
                    for i in range(n_pieces):
                        n = gbase + (1 if i < grem else 0)
                        bgroups.append((sec, w, tt0, n))
                        tt0 += n
            qb = _assign_queues([g[3] for g in bgroups])
            bgroups = [(sec, w, tt0, n, q) for (sec, w, tt0, n), q in zip(bgroups, qb)]
            psum_of_win = {}
            for sec, w, t0, n, qn in bgroups:
                g2 = g2pool.tile([CHUNK, GSZ, d_rep], BF16, tag="g2", name="g2b")
                view = h2tab[0:AVIEW, :] if sec == 0 else tab_view(h2tab)[1]
                nc.gpsimd.dma_gather(
                    g2[:, :n, :],
                    view,
                    idx_sb[:, t0 * 8 : (t0 + n) * 8],
                    n * CHUNK,
                    n * CHUNK,
                    d_rep,
                    single_packet=True,
                    queue_num=qn,
                )
                for bt0 in range(t0, t0 + n, SBATCH):
                    bn = min(SBATCH, t0 + n - bt0)
                    s_tile = build_s(s2pool, bt0, bn, "sB")
                    for t in range(bt0, bt0 + bn):
                        j = t - bt0
                        sec0_first = sec == 0 and t == chunk_base[w]
                        sec0_last = (
                            sec == 0 and t == chunk_base[w] + int(kw_lo_a[w]) - 1
                        )
                        sec1_first = (
                            sec == 1 and t == chunk_base[w] + int(kw_lo_a[w])
                        )
                        sec1_last = sec == 1 and t == chunk_base[w + 1] - 1
                        if sec0_first:
                            psum_of_win[w] = psA.tile(
                                [WINDOW, n_cls], F32, tag="agg", name="agg2ps"
                            )
                            h2w = selfpool.tile([WINDOW, d_rep], BF16, tag="h2w")
                            nc.sync.dma_start(
                                out=h2w[:],
                                in_=h2loc[w * WINDOW : (w + 1) * WINDOW, :],
                            )
                            nc.tensor.matmul(
                                out=psum_of_win[w][:],
                                lhsT=idb_sb[:],
                                rhs=h2w[:, :n_cls],
                                start=True,
                                stop=False,
                            )
                        elif sec1_first:
                            psum_of_win[w] = psA.tile(
                                [WINDOW, n_cls], F32, tag="agg", name="agg2ps"
                            )
                        nc.tensor.matmul(
                            out=psum_of_win[w][:],
                            lhsT=s_tile[:, j * WINDOW : (j + 1) * WINDOW],
                            rhs=g2[:, t - t0, :n_cls],
                            start=sec1_first,
                            stop=sec0_last or sec1_last,
                        )
                        osl = out2_sb[:, w * n_cls : (w + 1) * n_cls]
                        if sec0_last:
                            ps = psum_of_win.pop(w)
                            nc.scalar.activation(out=osl, in_=ps[:], func=Copy)
                        elif sec1_last:
                            ps = psum_of_win.pop(w)
                            ob = wpool.tile([WINDOW, n_cls], F32, tag="ob")
                            nc.vector.tensor_tensor(
                                out=ob[:], in0=ps[:], in1=osl,
                                op=mybir.AluOpType.add,
                            )
                            if bias2_zero:
                                nc.vector.tensor_scalar(
                                    out=osl,
                                    in0=ob[:],
                                    scalar1=dinvw_sb[:, w : w + 1],
                                    scalar2=None,
                                    op0=mybir.AluOpType.mult,
                                )
                            else:
                                ob2 = wpool.tile([WINDOW, n_cls], F32, tag="ob2")
                                nc.vector.tensor_scalar(
                                    out=ob2[:],
                                    in0=ob[:],
                                    scalar1=dinvw_sb[:, w : w + 1],
                                    scalar2=None,
                                    op0=mybir.AluOpType.mult,
                                )
                                nc.vector.tensor_tensor(
                                    out=osl, in0=ob2[:], in1=b2_sb[:],
                                    op=mybir.AluOpType.add,
                                )
            nc.sync.dma_start(out=out_t[:], in_=out2_sb[:])

    nc.compile()
    return nc


def _window_epilogue_A(
    nc, w, aggT, wpool, psW, w1_sb, w2_sb, b1_sb, dinvw_sb, d2_sb, id_sb,
    idb_sb, h2loc, n_local, d_in, d_hid, n_cls, d_rep, bias_zero,
):
    """aggT [d_in, WINDOW] in SBUF -> replicated h2 rows in DRAM."""
    Relu = mybir.ActivationFunctionType.Relu
    Copy = mybir.ActivationFunctionType.Copy

    # h1 [dst, hid] = aggT.T @ W1
    h1_ps = psW.tile([WINDOW, d_hid], F32, tag="wps", name="h1_ps")
    nc.tensor.matmul(out=h1_ps[:], lhsT=aggT, rhs=w1_sb[:], start=True, stop=True)
    if bias_zero:
        # b1 == 0: relu(dinv*x) = dinv*relu(x); relu straight off PSUM and
        # fold both dinv factors into the final replicate-scale (dinv^2).
        r3_sb = wpool.tile([WINDOW, d_hid], F32, tag="r3")
        nc.scalar.activation(out=r3_sb[:], in_=h1_ps[:], func=Relu)
        fin_scale = d2_sb
    else:
        r_sb = wpool.tile([WINDOW, d_hid], F32, tag="r")
        nc.vector.tensor_scalar(
            out=r_sb[:],
            in0=h1_ps[:],
            scalar1=dinvw_sb[:, w : w + 1],
            scalar2=None,
            op0=mybir.AluOpType.mult,
        )
        r2_sb = wpool.tile([WINDOW, d_hid], F32, tag="r2")
        nc.vector.tensor_tensor(
            out=r2_sb[:], in0=r_sb[:], in1=b1_sb[:], op=mybir.AluOpType.add
        )
        r3_sb = wpool.tile([WINDOW, d_hid], F32, tag="r3")
        nc.scalar.activation(out=r3_sb[:], in_=r2_sb[:], func=Relu)
        fin_scale = dinvw_sb
    # transpose -> [hid, dst]
    rT_ps = psW.tile([d_hid, WINDOW], F32, tag="wps", name="rT_ps")
    nc.tensor.transpose(out=rT_ps[:], in_=r3_sb[:], identity=id_sb[:])
    rT_sb = wpool.tile([d_hid, WINDOW], F32, tag="rTs")
    nc.scalar.activation(out=rT_sb[:], in_=rT_ps[:], func=Copy)
    # h2 [dst, n_cls] = rT.T @ W2; scale; replicate REP x
    h2_ps = psW.tile([WINDOW, n_cls], F32, tag="wps", name="h2_ps")
    nc.tensor.matmul(out=h2_ps[:], lhsT=rT_sb[:], rhs=w2_sb[:], start=True, stop=True)
    h2_sb = wpool.tile([WINDOW, d_rep], BF16, tag="h2s")
    nc.vector.tensor_scalar(
        out=h2_sb[:].rearrange("p (r c) -> p r c", c=n_cls),
        in0=h2_ps[:]
        .rearrange("p (one c) -> p one c", one=1)
        .to_broadcast([WINDOW, REP, n_cls]),
        scalar1=fin_scale[:, w : w + 1],
        scalar2=None,
        op0=mybir.AluOpType.mult,
    )
    nrows = min(WINDOW, n_local - w * WINDOW)
    nc.sync.dma_start(
        out=h2loc[w * WINDOW : w * WINDOW + nrows, :], in_=h2_sb[:nrows, :]
    )


# --------------------------------------------------------------------------
# Entry point
# --------------------------------------------------------------------------
def _make_inputs(x, W1, b1, W2, b2, pp, dt_np, n_cores):
    N, d_in = x.shape
    NP = n_cores * pp["n_local"]
    W1 = np.asarray(W1, np.float32)
    b1 = np.asarray(b1, np.float32)
    W2 = np.asarray(W2, np.float32)
    b2 = np.asarray(b2, np.float32)
    d_hid = W1.shape[1]
    n_cls = W2.shape[1]
    xp = x * pp["dinv"][:, None]
    xtab = np.zeros((NP + 2, d_in), np.float32)
    xtab[pp["tabrow_of_pos"]] = xp
    xtab = xtab.astype(dt_np)
    iota_arr = np.broadcast_to(
        np.tile(np.arange(WINDOW, dtype=np.float32), SBATCH),
        (CHUNK, SBATCH * WINDOW),
    ).astype(np.dtype("bfloat16"))
    shared = {
        "xtab": xtab,
        "w1": W1,
        "w2": W2,
        "b1bc": np.broadcast_to(b1, (WINDOW, d_hid)).astype(np.float32).copy(),
        "b2bc": np.broadcast_to(b2, (WINDOW, n_cls)).astype(np.float32).copy(),
        "iota": iota_arr,
        "ident": np.eye(WINDOW, dtype=np.float32),
        "identb": np.eye(WINDOW, dtype=np.float32).astype(dt_np),
    }
    n_local = pp["n_local"]
    in_maps = []
    for c, pc in enumerate(pp["per_core"]):
        m = dict(shared)
        xs = np.zeros((n_local, d_in), np.float32)
        sl = pp["inv"][c * n_local : (c + 1) * n_local]
        xs[sl >= 0] = xp[sl[sl >= 0]]
        m["xself"] = xs.astype(dt_np)
        m["idx16"] = pc["idx16"]
        m["dstrel"] = pc["dstrel"].astype(np.dtype("bfloat16"))
        m["dinvw"] = pc["dinvw"]
        in_maps.append(m)
    return in_maps


def _run(x, edge_index, W1, b1, W2, b2, n_cores, trace=False):
    x = np.asarray(x, dtype=np.float32)
    N, d_in = x.shape
    d_hid = np.asarray(W1).shape[1]
    n_cls = np.asarray(W2).shape[1]
    assert d_in == 128 and d_hid == 128

    pp = _preprocess(x, edge_index, n_cores)
    dt_gat = BF16 if GATHER_BF16 else F32
    np_gat = np.dtype("bfloat16") if GATHER_BF16 else np.dtype("float32")

    nc = bacc.Bacc("TRN2", target_bir_lowering=False, debug=False, num_swdge_queues=4)
    _build(
        nc,
        N=n_cores * pp["n_local"],
        n_local=pp["n_local"],
        d_in=d_in,
        d_hid=d_hid,
        n_cls=n_cls,
        pp=pp,
        n_cores=n_cores,
        dt_gat=dt_gat,
        bias_zero=bool(np.all(np.asarray(b1) == 0)),
        bias2_zero=bool(np.all(np.asarray(b2) == 0)),
    )

    import ml_dtypes  # noqa

    in_maps = _make_inputs(x, W1, b1, W2, b2, pp, np_gat, n_cores)
    res = run_bass_kernel_spmd(nc, in_maps, list(range(n_cores)), trace=trace)
    w_cnt = pp["w_cnt"]
    outs = [
        res.results[c]["out"]
        .reshape(WINDOW, w_cnt, 2)
        .transpose(1, 0, 2)
        .reshape(pp["n_local"], 2)
        for c in range(n_cores)
    ]
    full = np.concatenate(outs, axis=0)[pp["permpos"]]
    return full.astype(np.float32), res


def kernel(x, edge_index, W1, b1, W2, b2):
    out, _ = _run(x, edge_index, W1, b1, W2, b2, N_CORES)
    return out

